# revision 8
# baseline (speedup 1.0000x reference)
"""Trainium2 Bass kernel for nn_ComplexityDecoderLayer (moe_routing).

Strategy (8 NeuronCores, SPMD):
  - Token-parallel attention + PID dynamics: each core owns 256 of 2048 tokens.
    K/V are computed per-shard (with qk-norm + RoPE) and AllGathered packed as
    one [256, 512] -> [2048, 512] collective.
  - Attention uses exp without max-subtraction (scores are O(1) after qk-norm),
    computed in transposed layout [keys, q] so softmax numerator/denominator
    come out of the same PSUM accumulation via a ones-column appended to V.
  - Expert-parallel MoE: routing argmax on device, x + expert_id AllGathered,
    token dispatch/undispatch via 0/1 permutation matmuls built from a
    triangular-matmul prefix-sum (capacity CAP per expert), ReduceScatter
    returns each token's expert output to its owner core.
All heavy compute is fp32 on device; the host only slices/concats inputs,
precomputes RoPE cos/sin tables from `positions`, the base-id one-hot from
`token_ids`, and reassembles the three outputs.
"""

import numpy as np

import concourse.mybir as mybir
import concourse.tile as tile
from concourse import bacc
from concourse.bass_utils import run_bass_kernel_spmd

F32 = mybir.dt.float32
AF = mybir.ActivationFunctionType
OP = mybir.AluOpType
AX = mybir.AxisListType

P = 128
N, D, H, KV, DH, E, FF, CH = 2048, 1024, 16, 4, 64, 8, 2048, 64
NC_ = 8
NT = N // NC_          # 256 tokens per core
RT = NT // P           # 2 row tiles
DT_ = D // P           # 8
FT = FF // P           # 16
JT = N // P            # 16 global token tiles
EPS = 1e-6
THETA = 10000.0
DTC = 0.1
BASE_SCALE = 10.0

_CACHE = {}


def _build(CAP):
    ST = CAP // P
    assert ST <= 4
    nc = bacc.Bacc(target_bir_lowering=False)

    def par(name, shp):
        return nc.declare_dram_parameter(name, list(shp), F32, isOutput=False)

    hid_p = par("hid", [NT, D])
    mu_p = par("mu", [NT, D])
    vel_p = par("vel", [NT, D])
    cs_p = par("cs", [NT, 2 * 32])          # [cos | sin]
    boh_p = par("boh", [NT, E])             # BASE_SCALE * one_hot(token_ids % E)
    wq_p = par("wq", [D, D])
    wmq_p = par("wmq", [D, D])
    wk_p = par("wk", [D, KV * DH])
    wmk_p = par("wmk", [D, KV * DH])
    wv_p = par("wv", [D, KV * DH])
    wmv_p = par("wmv", [D, KV * DH])
    wo_p = par("wo", [D, D])
    dynw_p = par("dynw", [D, D])
    ciw_p = par("ciw", [2 * D, CH])
    cib_p = par("cib", [1, CH])
    cowx_p = par("cowx", [CH + 1, 3 * D])   # [ctrl_out_w ; ctrl_out_b]
    mrw_p = par("mrw", [D, E])
    wg_p = par("wg", [D, FF])               # this core's expert
    wu_p = par("wu", [D, FF])
    wd_p = par("wd", [FF, D])
    ln1_p = par("ln1", [1, D])
    ln2_p = par("ln2", [1, D])
    qnw_p = par("qnw", [1, D])              # qnorm_w tiled 16x
    knw_p = par("knw", [1, KV * DH])        # knorm_w tiled 4x
    dmu_p = par("dmu", [1, D])
    trib_p = par("trib", [P, P])            # strict upper triangular ones
    ident_p = par("ident", [P, P])
    iotac_p = par("iotac", [1, CAP])
    iota8_p = par("iota8", [1, E])
    mye_p = par("mye", [P, 1])              # this core's expert index (f32)

    oh_p = nc.declare_dram_parameter("oh", [NT, D], F32, isOutput=True)
    ov_p = nc.declare_dram_parameter("ov", [NT, D], F32, isOutput=True)
    om_p = nc.declare_dram_parameter("om", [NT, D], F32, isOutput=True)

    with tile.TileContext(nc) as tc:
        from contextlib import ExitStack
        with ExitStack() as TOP:
            dram = TOP.enter_context(tc.tile_pool(name="dram", bufs=1, space="DRAM"))
            const = TOP.enter_context(tc.tile_pool(name="const", bufs=1))
            ps = TOP.enter_context(tc.tile_pool(name="ps", bufs=1, space="PSUM"))
            ws = TOP.enter_context(tc.tile_pool(name="wstream", bufs=1))
            work = TOP.enter_context(tc.tile_pool(name="work", bufs=1))
            top = TOP.enter_context(tc.tile_pool(name="top", bufs=1))

            # phase-scoped pools (manually closed LIFO per side to free SBUF)
            cm_dyn = tc.tile_pool(name="p_dyn", bufs=1); p_dyn = cm_dyn.__enter__()
            cm_att = tc.tile_pool(name="p_att", bufs=1); p_att = cm_att.__enter__()
            cm_hm = tc.tile_pool(name="p_hm", bufs=1); p_hm = cm_hm.__enter__()

            # ---------------- constants ----------------
            ident = const.tile([P, P], F32, name="identc")
            nc.sync.dma_start(out=ident[:, :], in_=ident_p[:, :])
            trib = const.tile([P, P], F32, name="tribc")
            nc.sync.dma_start(out=trib[:, :], in_=trib_p[:, :])
            ln1b = p_dyn.tile([P, D], F32, name="ln1b")
            nc.sync.dma_start(out=ln1b[:, :], in_=ln1_p[:, :].to_broadcast((P, D)))
            ln2b = p_dyn.tile([P, D], F32, name="ln2b")
            nc.sync.dma_start(out=ln2b[:, :], in_=ln2_p[:, :].to_broadcast((P, D)))
            qnwb = p_dyn.tile([P, D], F32, name="qnwb")
            nc.sync.dma_start(out=qnwb[:, :], in_=qnw_p[:, :].to_broadcast((P, D)))
            knwb = p_dyn.tile([P, KV * DH], F32, name="knwb")
            nc.sync.dma_start(out=knwb[:, :], in_=knw_p[:, :].to_broadcast((P, KV * DH)))
            iota8b = const.tile([P, E], F32, name="iota8b")
            nc.sync.dma_start(out=iota8b[:, :], in_=iota8_p[:, :].to_broadcast((P, E)))
            mye = const.tile([P, 1], F32, name="myec")
            nc.sync.dma_start(out=mye[:, :], in_=mye_p[:, :])
            epsb = const.tile([P, 1], F32, name="epsb")
            nc.vector.memset(epsb[:, :], EPS)
            ones_r = const.tile([1, NT], F32, name="onesr")   # lhsT row for bias matmuls
            nc.vector.memset(ones_r[:, :], 1.0)
            ones_c = const.tile([P, 1], F32, name="onesc")    # rhs col for colsum
            nc.vector.memset(ones_c[:, :], 1.0)
            dmu_sb = const.tile([1, D], F32, name="dmusb")
            nc.sync.dma_start(out=dmu_sb[:, :], in_=dmu_p[:, :])
            cib_sb = const.tile([1, CH], F32, name="cibsb")
            nc.sync.dma_start(out=cib_sb[:, :], in_=cib_p[:, :])
            ciw_sb = p_dyn.tile([P, 16 * CH], F32, name="ciwsb")  # [2048,64] -> [128, 16*64]
            nc.sync.dma_start(
                out=ciw_sb[:, :].rearrange("p (j c) -> p j c", j=16),
                in_=ciw_p[:, :].rearrange("(j p) c -> p j c", p=P),
            )
            mrw_sb = p_dyn.tile([P, DT_ * E], F32, name="mrwsb")  # [1024,8] -> [128, 8*8]
            nc.sync.dma_start(
                out=mrw_sb[:, :].rearrange("p (j c) -> p j c", j=DT_),
                in_=mrw_p[:, :].rearrange("(j p) c -> p j c", p=P),
            )
            cos_sb = [p_dyn.tile([P, 32], F32, name=f"cos{rt}") for rt in range(RT)]
            sin_sb = [p_dyn.tile([P, 32], F32, name=f"sin{rt}") for rt in range(RT)]
            for rt in range(RT):
                nc.sync.dma_start(out=cos_sb[rt][:, :], in_=cs_p[rt * P:(rt + 1) * P, 0:32])
                nc.sync.dma_start(out=sin_sb[rt][:, :], in_=cs_p[rt * P:(rt + 1) * P, 32:64])

            # ---------------- DRAM internals ----------------
            kv_in = dram.tile([NT, 2 * KV * DH], F32, name="kvin")
            kv_full = dram.tile([N, 2 * KV * DH], F32, name="kvfull", addr_space="Shared")
            xe_in = dram.tile([NT, D + 8], F32, name="xein")
            xe_full = dram.tile([N, D + 8], F32, name="xefull", addr_space="Shared")
            y_loc = dram.tile([N, D], F32, name="yloc")
            y_rows = dram.tile([NT, D], F32, name="yrows")

            def peT(src_ap, dst_ap, engine):
                """dst = src^T via PE transpose (src [p, f] -> dst [f, p])."""
                f = src_ap.shape[-1]
                p_ = src_ap.shape[0]
                pt = ps.tile([P, P], F32, tag="pt", bufs=2, name="pt")
                nc.tensor.transpose(pt[0:f, 0:p_], src_ap, ident[0:p_, 0:p_])
                engine(dst_ap, pt[0:f, 0:p_])

            vcopy = nc.vector.tensor_copy
            scopy = nc.scalar.copy

            def rmsnorm(dst, src, wb, ddim):
                """dst = src * rsqrt(mean(src^2)+eps) * wb  (row layout)."""
                t = work.tile([P, ddim], F32, tag="wk1024", bufs=4, name="rmst")
                sS = work.tile([P, 1], F32, tag="rms_s", bufs=4, name="rmss")
                nc.scalar.activation(t[:, 0:ddim], src, AF.Square, accum_out=sS[:, :])
                sq = work.tile([P, 1], F32, tag="rms_q", bufs=4, name="rmsq")
                nc.scalar.activation(sq[:, :], sS[:, :], AF.Sqrt, bias=epsb[:, :], scale=1.0 / ddim)
                rs_ = work.tile([P, 1], F32, tag="rms_r", bufs=4, name="rmsr")
                nc.vector.reciprocal(rs_[:, :], sq[:, :])
                nc.vector.tensor_scalar_mul(dst, src, rs_[:, :])
                nc.vector.tensor_tensor(dst, dst, wb, OP.mult)

            def headnorm(qr, nh, wb):
                """per-64-col rmsnorm in rows layout, then * wb (tiled weight bcast)."""
                for hh in range(nh):
                    sl = qr[:, hh * DH:(hh + 1) * DH]
                    t = work.tile([P, DH], F32, tag="hn_t", bufs=2, name="hnt")
                    sS = work.tile([P, 1], F32, tag="hn_s", bufs=4, name="hns")
                    nc.scalar.activation(t[:, :], sl, AF.Square, accum_out=sS[:, :])
                    sq = work.tile([P, 1], F32, tag="hn_q", bufs=4, name="hnq")
                    nc.scalar.activation(sq[:, :], sS[:, :], AF.Sqrt, bias=epsb[:, :], scale=1.0 / DH)
                    rs_ = work.tile([P, 1], F32, tag="hn_r", bufs=4, name="hnr")
                    nc.vector.reciprocal(rs_[:, :], sq[:, :])
                    nc.vector.tensor_scalar_mul(sl, sl, rs_[:, :])
                nc.vector.tensor_tensor(qr, qr, wb[:, 0:qr.shape[-1]], OP.mult)

            def rope(dst, src, rt, nh):
                """NeoX rotate in rows layout; src/dst [P, nh*64]."""
                s3 = src.rearrange("p (h d) -> p h d", h=nh)
                d3 = dst.rearrange("p (h d) -> p h d", h=nh)
                c3 = cos_sb[rt][:, :].rearrange("p (o d) -> p o d", o=1).to_broadcast((P, nh, 32))
                n3 = sin_sb[rt][:, :].rearrange("p (o d) -> p o d", o=1).to_broadcast((P, nh, 32))
                tmp = work.tile([P, H * 32], F32, tag="rope_t", bufs=2, name="ropet")
                t3 = tmp[:, 0:nh * 32].rearrange("p (h d) -> p h d", h=nh)
                x1 = s3[:, :, 0:32]
                x2 = s3[:, :, 32:64]
                nc.vector.tensor_tensor(d3[:, :, 0:32], x1, c3, OP.mult)
                nc.vector.tensor_tensor(t3, x2, n3, OP.mult)
                nc.vector.tensor_tensor(d3[:, :, 0:32], d3[:, :, 0:32], t3, OP.subtract)
                nc.vector.tensor_tensor(d3[:, :, 32:64], x2, c3, OP.mult)
                nc.vector.tensor_tensor(t3, x1, n3, OP.mult)
                nc.vector.tensor_tensor(d3[:, :, 32:64], d3[:, :, 32:64], t3, OP.add)

            # ================= Phase 1: norms, projections, rope =================
            hid = [p_dyn.tile([P, D], F32, name=f"hid{rt}") for rt in range(RT)]
            vel = [p_dyn.tile([P, D], F32, name=f"vel{rt}") for rt in range(RT)]
            velT = [p_dyn.tile([P, NT], F32, name=f"velT{k}") for k in range(DT_)]
            hT = [p_hm.tile([P, NT], F32, name=f"hT{k}") for k in range(DT_)]
            muT = [p_hm.tile([P, NT], F32, name=f"muT{k}") for k in range(DT_)]
            qrows = [p_hm.tile([P, D], F32, name=f"qrows{rt}") for rt in range(RT)]
            h2 = [top.tile([P, D], F32, name=f"h2{rt}") for rt in range(RT)]

            for rt in range(RT):
                nc.sync.dma_start(out=hid[rt][:, :], in_=hid_p[rt * P:(rt + 1) * P, :])
                nc.sync.dma_start(out=vel[rt][:, :], in_=vel_p[rt * P:(rt + 1) * P, :])
                h = work.tile([P, D], F32, tag="wk1024", bufs=4, name="hrows")
                rmsnorm(h[:, :], hid[rt][:, :], ln1b[:, :], D)
                mrow = work.tile([P, D], F32, tag="wk1024", bufs=4, name="murows")
                nc.sync.dma_start(out=mrow[:, :], in_=mu_p[rt * P:(rt + 1) * P, :])
                for k in range(DT_):
                    peT(h[:, k * P:(k + 1) * P], hT[k][:, rt * P:(rt + 1) * P], vcopy)
                    peT(mrow[:, k * P:(k + 1) * P], muT[k][:, rt * P:(rt + 1) * P], vcopy)
                    peT(vel[rt][:, k * P:(k + 1) * P], velT[k][:, rt * P:(rt + 1) * P], vcopy)

            # q rows = h @ wq + mu @ wmq
            for nt in range(2):
                pq = [ps.tile([P, 512], F32, tag="big", bufs=4, name="pq") for _ in range(RT)]
                i = 0
                for lhsT, w_p in ((hT, wq_p), (muT, wmq_p)):
                    for k in range(DT_):
                        wt = ws.tile([P, 512], F32, tag="w512", bufs=4, name="wt")
                        nc.sync.dma_start(out=wt[:, :], in_=w_p[k * P:(k + 1) * P, nt * 512:(nt + 1) * 512])
                        for rt in range(RT):
                            nc.tensor.matmul(pq[rt][:, :], lhsT[k][:, rt * P:(rt + 1) * P], wt[:, :],
                                             start=(i == 0), stop=(i == 2 * DT_ - 1))
                        i += 1
                for rt in range(RT):
                    vcopy(qrows[rt][:, nt * 512:(nt + 1) * 512], pq[rt][:, :])

            # k/v rows (this shard's 4 kv heads)
            for rt in range(RT):
                pk = ps.tile([P, KV * DH], F32, tag="big", bufs=4, name="pk")
                pv = ps.tile([P, KV * DH], F32, tag="big", bufs=4, name="pv")
                i = 0
                for lhsT, wp1, wp2 in ((hT, wk_p, wv_p), (muT, wmk_p, wmv_p)):
                    for k in range(DT_):
                        wt1 = ws.tile([P, KV * DH], F32, tag="w256", bufs=4, name="wt1")
                        nc.sync.dma_start(out=wt1[:, :], in_=wp1[k * P:(k + 1) * P, :])
                        wt2 = ws.tile([P, KV * DH], F32, tag="w256", bufs=4, name="wt2")
                        nc.sync.dma_start(out=wt2[:, :], in_=wp2[k * P:(k + 1) * P, :])
                        nc.tensor.matmul(pk[:, :], lhsT[k][:, rt * P:(rt + 1) * P],
                                         wt1[:, :], start=(i == 0), stop=(i == 2 * DT_ - 1))
                        nc.tensor.matmul(pv[:, :], lhsT[k][:, rt * P:(rt + 1) * P],
                                         wt2[:, :], start=(i == 0), stop=(i == 2 * DT_ - 1))
                        i += 1
                krow = p_hm.tile([P, KV * DH], F32, tag="kv256", bufs=3, name="krow")
                vrow = p_hm.tile([P, KV * DH], F32, tag="kv256b", bufs=3, name="vrow")
                vcopy(krow[:, :], pk[:, :])
                vcopy(vrow[:, :], pv[:, :])
                headnorm(krow[:, :], KV, knwb)
                rk = p_hm.tile([P, KV * DH], F32, tag="kv256c", bufs=3, name="rk")
                rope(rk[:, :], krow[:, :], rt, KV)
                nc.sync.dma_start(out=kv_in[rt * P:(rt + 1) * P, 0:KV * DH], in_=rk[:, :])
                nc.sync.dma_start(out=kv_in[rt * P:(rt + 1) * P, KV * DH:2 * KV * DH], in_=vrow[:, :])

            qT = [p_att.tile([DH, NT], F32, name=f"qT{hh}") for hh in range(H)]
            for rt in range(RT):
                headnorm(qrows[rt][:, :], H, qnwb)
                rq = work.tile([P, D], F32, tag="wk1024", bufs=4, name="rq")
                rope(rq[:, :], qrows[rt][:, :], rt, H)
                for k in range(DT_):
                    pt = ps.tile([P, P], F32, tag="pt", bufs=2, name="ptq")
                    nc.tensor.transpose(pt[:, :], rq[:, k * P:(k + 1) * P], ident[:, :])
                    vcopy(qT[2 * k][:, rt * P:(rt + 1) * P], pt[0:DH, :])
                    vcopy(qT[2 * k + 1][:, rt * P:(rt + 1) * P], pt[DH:P, :])

            cm_hm.__exit__(None, None, None)  # free hT/muT/qrows

            # ================= Phase 2: AllGather k/v =================
            nc.gpsimd.collective_compute(
                "AllGather", OP.bypass, replica_groups=[list(range(NC_))],
                ins=[kv_in[:, :].opt()], outs=[kv_full[:, :].opt()],
            )

            kT = [p_att.tile([DH, N], F32, name=f"kT{g}") for g in range(KV)]
            vext4 = [p_att.tile([P, JT * 65], F32, name=f"vext{g}") for g in range(KV)]
            for g in range(KV):
                nc.vector.memset(vext4[g][:, :], 1.0)
            for tt in range(JT):
                kl = p_att.tile([P, KV * DH], F32, tag="kl", bufs=2, name="kl")
                nc.sync.dma_start(out=kl[:, :], in_=kv_full[tt * P:(tt + 1) * P, 0:KV * DH])
                for half in range(2):
                    pt = ps.tile([P, P], F32, tag="pt", bufs=2, name="ptk")
                    nc.tensor.transpose(pt[:, :], kl[:, half * P:(half + 1) * P], ident[:, :])
                    vcopy(kT[2 * half][:, tt * P:(tt + 1) * P], pt[0:DH, :])
                    vcopy(kT[2 * half + 1][:, tt * P:(tt + 1) * P], pt[DH:P, :])
                for g in range(KV):
                    nc.sync.dma_start(out=vext4[g][:, tt * 65:tt * 65 + 64],
                                      in_=kv_full[tt * P:(tt + 1) * P, KV * DH + g * DH:KV * DH + (g + 1) * DH])

            # ================= Phase 3: attention =================
            cm_o = tc.tile_pool(name="p_o", bufs=1, side="right"); p_o = cm_o.__enter__()
            oT = [p_o.tile([P, NT], F32, name=f"oT{k}") for k in range(DT_)]
            for g in range(KV):
                vext = vext4[g]
                for hq in range(H // KV):
                    hh = g * (H // KV) + hq
                    qTh = qT[hh][:, :]
                    pO = ps.tile([65, NT], F32, tag="oext", bufs=2, name="pO")
                    for tt in range(JT):
                        pS = ps.tile([P, NT], F32, tag="big", bufs=4, name="pS")
                        nc.tensor.matmul(pS[:, :], kT[g][:, tt * P:(tt + 1) * P],
                                         qTh, start=True, stop=True)
                        ex = p_att.tile([P, NT], F32, tag="ex", bufs=3, name="ex")
                        nc.scalar.activation(ex[:, :], pS[:, :], AF.Exp, scale=0.125)
                        nc.tensor.matmul(pO[:, :], vext[:, tt * 65:(tt + 1) * 65], ex[:, :],
                                         start=(tt == 0), stop=(tt == JT - 1))
                    rd = p_att.tile([1, NT], F32, tag="rd", bufs=2, name="rd")
                    nc.vector.reciprocal(rd[:, :], pO[64:65, :])
                    dbn = dram.tile([1, NT], F32, tag="dbn", bufs=2, name="dbn")
                    nc.sync.dma_start(out=dbn[:, :], in_=rd[:, :])
                    rdb = p_att.tile([DH, NT], F32, tag="rdb", bufs=2, name="rdb")
                    nc.sync.dma_start(out=rdb[:, :], in_=dbn[:, :].to_broadcast((DH, NT)))
                    nc.vector.tensor_tensor(oT[hh // 2][(hh % 2) * DH:(hh % 2 + 1) * DH, :],
                                            pO[0:DH, :], rdb[:, :], OP.mult)

            cm_att.__exit__(None, None, None)  # free qT/kT/v_sb/vext/ex

            # ================= Phase 4: wo + dynamics + router =================
            cm_wo = tc.tile_pool(name="p_wo", bufs=1); p_wo = cm_wo.__enter__()
            orows = [p_wo.tile([P, D], F32, name=f"orows{rt}") for rt in range(RT)]
            for nt in range(2):
                po = [ps.tile([P, 512], F32, tag="big", bufs=4, name="po") for _ in range(RT)]
                for k in range(DT_):
                    wt = ws.tile([P, 512], F32, tag="w512", bufs=4, name="wot")
                    nc.sync.dma_start(out=wt[:, :], in_=wo_p[k * P:(k + 1) * P, nt * 512:(nt + 1) * 512])
                    for rt in range(RT):
                        nc.tensor.matmul(po[rt][:, :], oT[k][:, rt * P:(rt + 1) * P], wt[:, :],
                                         start=(k == 0), stop=(k == DT_ - 1))
                for rt in range(RT):
                    vcopy(orows[rt][:, nt * 512:(nt + 1) * 512], po[rt][:, :])

            oTw = [p_wo.tile([P, NT], F32, name=f"oTw{k}") for k in range(DT_)]
            for rt in range(RT):
                for k in range(DT_):
                    peT(orows[rt][:, k * P:(k + 1) * P], oTw[k][:, rt * P:(rt + 1) * P], vcopy)
            cm_o.__exit__(None, None, None)  # free oT

            # mu_cur = dyn_mu + o @ dynw
            mucur = [p_wo.tile([P, D], F32, name=f"mucur{rt}") for rt in range(RT)]
            for nt in range(2):
                pm = [ps.tile([P, 512], F32, tag="big", bufs=4, name="pm") for _ in range(RT)]
                for k in range(DT_):
                    wt = ws.tile([P, 512], F32, tag="w512", bufs=4, name="dynt")
                    nc.sync.dma_start(out=wt[:, :], in_=dynw_p[k * P:(k + 1) * P, nt * 512:(nt + 1) * 512])
                    for rt in range(RT):
                        nc.tensor.matmul(pm[rt][:, :], oTw[k][:, rt * P:(rt + 1) * P], wt[:, :],
                                         start=(k == 0), stop=False)
                for rt in range(RT):
                    nc.tensor.matmul(pm[rt][:, :], ones_r[0:1, rt * P:(rt + 1) * P],
                                     dmu_sb[0:1, nt * 512:(nt + 1) * 512], start=False, stop=True)
                    vcopy(mucur[rt][:, nt * 512:(nt + 1) * 512], pm[rt][:, :])
            for rt in range(RT):
                nc.sync.dma_start(out=om_p[rt * P:(rt + 1) * P, :], in_=mucur[rt][:, :])

            # ctrl MLP
            ctT = p_wo.tile([CH + 1, NT], F32, name="ctT")
            nc.vector.memset(ctT[CH:CH + 1, :], 1.0)
            for rt in range(RT):
                pc = ps.tile([P, CH], F32, tag="big", bufs=4, name="pc")
                for k in range(DT_):
                    nc.tensor.matmul(pc[:, :], oTw[k][:, rt * P:(rt + 1) * P],
                                     ciw_sb[:, k * CH:(k + 1) * CH], start=(k == 0), stop=False)
                for k in range(DT_):
                    nc.tensor.matmul(pc[:, :], velT[k][:, rt * P:(rt + 1) * P],
                                     ciw_sb[:, (DT_ + k) * CH:(DT_ + k + 1) * CH], start=False, stop=False)
                nc.tensor.matmul(pc[:, :], ones_r[0:1, rt * P:(rt + 1) * P], cib_sb[0:1, :],
                                 start=False, stop=True)
                ct = work.tile([P, CH], F32, tag="ct", bufs=2, name="ct")
                nc.scalar.activation(ct[:, :], pc[:, :], AF.Silu)
                peT(ct[:, :], ctT[0:CH, rt * P:(rt + 1) * P], vcopy)

            abg = [[p_wo.tile([P, D], F32, name=f"abg{i}{rt}") for rt in range(RT)] for i in range(3)]
            for nt in (0, 1, 4, 5, 2, 3):
                cw = ws.tile([CH + 1, 512], F32, tag="cow", bufs=3, name="cw")
                nc.sync.dma_start(out=cw[:, :], in_=cowx_p[:, nt * 512:(nt + 1) * 512])
                for rt in range(RT):
                    pb = ps.tile([P, 512], F32, tag="big", bufs=4, name="pb")
                    nc.tensor.matmul(pb[:, :], ctT[:, rt * P:(rt + 1) * P], cw[:, :],
                                     start=True, stop=True)
                    dst = abg[nt // 2][rt][:, (nt % 2) * 512:(nt % 2 + 1) * 512]
                    if nt // 2 != 1:
                        nc.scalar.activation(dst, pb[:, :], AF.Sigmoid)
                    else:
                        # softplus = ln(1 + exp(x)); Exp and Ln share one ACT table.
                        # exp overflow -> inf -> ln -> inf -> min(.,2) still correct.
                        nc.scalar.activation(dst, pb[:, :], AF.Exp)
                        nc.vector.tensor_scalar_add(dst, dst, 1.0)
                        nc.scalar.activation(dst, dst, AF.Ln)
            for rt in range(RT):
                nc.vector.tensor_scalar_min(abg[1][rt][:, :], abg[1][rt][:, :], 2.0)

            # dynamics elementwise + x
            for rt in range(RT):
                err = work.tile([P, D], F32, tag="wk1024", bufs=4, name="err")
                nc.vector.tensor_tensor(err[:, :], orows[rt][:, :], mucur[rt][:, :], OP.subtract)
                av = work.tile([P, D], F32, tag="wk1024", bufs=4, name="av")
                nc.vector.tensor_tensor(av[:, :], abg[0][rt][:, :], vel[rt][:, :], OP.mult)
                nc.vector.tensor_tensor(err[:, :], abg[1][rt][:, :], err[:, :], OP.mult)
                nc.vector.tensor_tensor(av[:, :], av[:, :], err[:, :], OP.subtract)
                nc.vector.tensor_scalar_min(av[:, :], av[:, :], 10.0)
                nc.vector.tensor_scalar_max(av[:, :], av[:, :], -10.0)
                nc.sync.dma_start(out=ov_p[rt * P:(rt + 1) * P, :], in_=av[:, :])
                gv = work.tile([P, D], F32, tag="wk1024", bufs=4, name="gv")
                nc.vector.tensor_tensor(gv[:, :], abg[2][rt][:, :], av[:, :], OP.mult)
                nc.vector.tensor_scalar_mul(gv[:, :], gv[:, :], DTC)
                nc.vector.tensor_tensor(gv[:, :], gv[:, :], orows[rt][:, :], OP.add)
                nc.vector.tensor_tensor(h2[rt][:, :], gv[:, :], hid[rt][:, :], OP.add)
                xr = work.tile([P, D], F32, tag="wk1024", bufs=4, name="xr")
                rmsnorm(xr[:, :], h2[rt][:, :], ln2b[:, :], D)
                nc.sync.dma_start(out=xe_in[rt * P:(rt + 1) * P, 0:D], in_=xr[:, :])

            # router: logits = mu_cur @ mrw (+ base one-hot), argmax -> eid
            mcT = [p_wo.tile([P, NT], F32, name=f"mcT{k}") for k in range(DT_)]
            for rt in range(RT):
                for k in range(DT_):
                    peT(mucur[rt][:, k * P:(k + 1) * P], mcT[k][:, rt * P:(rt + 1) * P], vcopy)
            for rt in range(RT):
                pr = ps.tile([P, E], F32, tag="big", bufs=4, name="pr")
                for k in range(DT_):
                    nc.tensor.matmul(pr[:, :], mcT[k][:, rt * P:(rt + 1) * P],
                                     mrw_sb[:, k * E:(k + 1) * E], start=(k == 0), stop=(k == DT_ - 1))
                cmb = work.tile([P, E], F32, tag="cmb", bufs=2, name="cmb")
                bohs = work.tile([P, E], F32, tag="bohs", bufs=2, name="bohs")
                nc.sync.dma_start(out=bohs[:, :], in_=boh_p[rt * P:(rt + 1) * P, :])
                nc.vector.tensor_tensor(cmb[:, :], pr[:, :], bohs[:, :], OP.add)
                mx = work.tile([P, 1], F32, tag="mx", bufs=2, name="mx")
                nc.vector.reduce_max(mx[:, :], cmb[:, :], axis=AX.X)
                nc.vector.tensor_scalar(cmb[:, :], cmb[:, :], mx[:, :], None, OP.is_equal)
                nc.vector.tensor_tensor(cmb[:, :], cmb[:, :], iota8b[:, :], OP.mult)
                eidt = work.tile([P, 1], F32, tag="eidt", bufs=2, name="eidt")
                nc.vector.reduce_sum(eidt[:, :], cmb[:, :], axis=AX.X)
                nc.sync.dma_start(out=xe_in[rt * P:(rt + 1) * P, D:D + 1], in_=eidt[:, :])

            cm_wo.__exit__(None, None, None)   # free orows/oTw/mucur/ctT/abg/mcT
            cm_dyn.__exit__(None, None, None)  # free hid/vel/velT

            # ================= Phase 5: AllGather x/eid; MoE =================
            nc.gpsimd.collective_compute(
                "AllGather", OP.bypass, replica_groups=[list(range(NC_))],
                ins=[xe_in[:, :].opt()], outs=[xe_full[:, :].opt()],
            )

            cm_moe = tc.tile_pool(name="p_moe", bufs=1); p_moe = cm_moe.__enter__()
            iotacb = p_moe.tile([P, CAP], F32, name="iotacb")
            nc.sync.dma_start(out=iotacb[:, :], in_=iotac_p[:, :].to_broadcast((P, CAP)))
            eid_sb = p_moe.tile([P, JT], F32, name="eidsb")
            nc.sync.dma_start(
                out=eid_sb[:, :],
                in_=xe_full[:, D:D + 1].rearrange("(j p) o -> p (j o)", p=P),
            )
            mask = p_moe.tile([P, JT], F32, name="masksb")
            nc.vector.tensor_scalar(mask[:, :], eid_sb[:, :], mye[:, :], None, OP.is_equal)
            # rank = strict-prefix-sum of mask in token order (p minor within col j)
            pcs = ps.tile([JT, 1], F32, tag="pt", bufs=2, name="pcs")
            nc.tensor.matmul(pcs[:, :], mask[:, :], ones_c[:, :], start=True, stop=True)
            cs16 = p_moe.tile([JT, 1], F32, name="cs16")
            vcopy(cs16[:, :], pcs[:, :])
            pcp = ps.tile([JT, 1], F32, tag="pt", bufs=2, name="pcp")
            nc.tensor.matmul(pcp[:, :], trib[0:JT, 0:JT], cs16[:, :], start=True, stop=True)
            cp16 = p_moe.tile([JT, 1], F32, name="cp16")
            vcopy(cp16[:, :], pcp[:, :])
            cpr = p_moe.tile([1, JT], F32, name="cpr")
            peT(cp16[:, :], cpr[:, :], vcopy)
            cprb = p_moe.tile([P, JT], F32, name="cprb")
            nc.gpsimd.partition_broadcast(cprb[:, :], cpr[:, :])
            pex = ps.tile([P, JT], F32, tag="pt", bufs=2, name="pex")
            nc.tensor.matmul(pex[:, :], trib[:, :], mask[:, :], start=True, stop=True)
            rank = p_moe.tile([P, JT], F32, name="ranksb")
            nc.vector.tensor_tensor(rank[:, :], pex[:, :], cprb[:, :], OP.add)

            Pt = [p_moe.tile([P, N], F32, name=f"Pm{sm}") for sm in range(ST)]
            xsT = [p_moe.tile([P, CAP], F32, name=f"xsT{k}") for k in range(DT_)]
            cm_pt = tc.tile_pool(name="ptpool", bufs=1); ptpool = cm_pt.__enter__()
            PTt = [ptpool.tile([P, CAP], F32, name=f"PT{j}") for j in range(JT)]
            for j in range(JT):
                nc.vector.tensor_scalar(PTt[j][:, :], iotacb[:, :], rank[:, j:j + 1],
                                        mask[:, j:j + 1], OP.is_equal, OP.mult)
            # P = PT^T (for undispatch)
            for j in range(JT):
                for sm in range(ST):
                    peT(PTt[j][:, sm * P:(sm + 1) * P], Pt[sm][:, j * P:(j + 1) * P], scopy)
            # dispatch: x_selT[d, s] = sum_t x[t, d] * PT[t, s]
            for dg in range(2):
                pxs = [ps.tile([P, CAP], F32, tag="big", bufs=4, name="pxs") for _ in range(4)]
                for j in range(JT):
                    xf = p_moe.tile([P, 512], F32, tag="xf", bufs=4, name="xf")
                    nc.sync.dma_start(out=xf[:, :], in_=xe_full[j * P:(j + 1) * P, dg * 512:(dg + 1) * 512])
                    for dm in range(4):
                        nc.tensor.matmul(pxs[dm][:, :], xf[:, dm * P:(dm + 1) * P], PTt[j][:, :],
                                         start=(j == 0), stop=(j == JT - 1))
                for dm in range(4):
                    scopy(xsT[dg * 4 + dm][:, :], pxs[dm][:, :])
            cm_pt.__exit__(None, None, None)  # free PT

            # expert FFN (transposed): gT/uT [FF, CAP] tiles
            midT = [p_moe.tile([P, CAP], F32, name=f"midT{f}") for f in range(FT)]
            for fg in range(8):
                pg = [ps.tile([P, CAP], F32, tag="big", bufs=4, name="pg") for _ in range(2)]
                pu = [ps.tile([P, CAP], F32, tag="big", bufs=4, name="pu") for _ in range(2)]
                for k in range(DT_):
                    wgt = ws.tile([P, 256], F32, tag="w256", bufs=4, name="wgt")
                    nc.sync.dma_start(out=wgt[:, :], in_=wg_p[k * P:(k + 1) * P, fg * 256:(fg + 1) * 256])
                    wut = ws.tile([P, 256], F32, tag="w256", bufs=4, name="wut")
                    nc.sync.dma_start(out=wut[:, :], in_=wu_p[k * P:(k + 1) * P, fg * 256:(fg + 1) * 256])
                    for fm in range(2):
                        nc.tensor.matmul(pg[fm][:, :], wgt[:, fm * P:(fm + 1) * P], xsT[k][:, :],
                                         start=(k == 0), stop=(k == DT_ - 1))
                        nc.tensor.matmul(pu[fm][:, :], wut[:, fm * P:(fm + 1) * P], xsT[k][:, :],
                                         start=(k == 0), stop=(k == DT_ - 1))
                for fm in range(2):
                    gs = p_moe.tile([P, CAP], F32, tag="gs", bufs=2, name="gs")
                    nc.scalar.activation(gs[:, :], pg[fm][:, :], AF.Silu)
                    nc.vector.tensor_tensor(midT[fg * 2 + fm][:, :], gs[:, :], pu[fm][:, :], OP.mult)

            # down: y_sel [CAP, D]
            ysel = [p_moe.tile([P, D], F32, name=f"ysel{sm}") for sm in range(ST)]
            for nt in range(2):
                pd = [ps.tile([P, 512], F32, tag="big", bufs=4, name="pd") for _ in range(ST)]
                for k in range(FT):
                    wdt = ws.tile([P, 512], F32, tag="w512", bufs=4, name="wdt")
                    nc.sync.dma_start(out=wdt[:, :], in_=wd_p[k * P:(k + 1) * P, nt * 512:(nt + 1) * 512])
                    for sm in range(ST):
                        nc.tensor.matmul(pd[sm][:, :], midT[k][:, sm * P:(sm + 1) * P], wdt[:, :],
                                         start=(k == 0), stop=(k == FT - 1))
                for sm in range(ST):
                    scopy(ysel[sm][:, nt * 512:(nt + 1) * 512], pd[sm][:, :])

            # undispatch: y_local[t, d] = sum_s P[s, t] y_sel[s, d]
            for j in range(JT):
                for nt in range(2):
                    py = ps.tile([P, 512], F32, tag="big", bufs=4, name="py")
                    for sm in range(ST):
                        nc.tensor.matmul(py[:, :], Pt[sm][:, j * P:(j + 1) * P],
                                         ysel[sm][:, nt * 512:(nt + 1) * 512],
                                         start=(sm == 0), stop=(sm == ST - 1))
                    yl = p_moe.tile([P, 512], F32, tag="yl", bufs=4, name="yl")
                    scopy(yl[:, :], py[:, :])
                    nc.sync.dma_start(out=y_loc[j * P:(j + 1) * P, nt * 512:(nt + 1) * 512], in_=yl[:, :])

            nc.gpsimd.collective_compute(
                "ReduceScatter", OP.add, replica_groups=[list(range(NC_))],
                ins=[y_loc[:, :].opt()], outs=[y_rows[:, :].opt()],
            )

            for rt in range(RT):
                yr = work.tile([P, D], F32, tag="wk1024", bufs=4, name="yr")
                nc.sync.dma_start(out=yr[:, :], in_=y_rows[rt * P:(rt + 1) * P, :])
                nc.vector.tensor_tensor(yr[:, :], yr[:, :], h2[rt][:, :], OP.add)
                nc.sync.dma_start(out=oh_p[rt * P:(rt + 1) * P, :], in_=yr[:, :])

            cm_moe.__exit__(None, None, None)

    nc.finalize()
    return nc


def _get_nc(CAP):
    if CAP not in _CACHE:
        _CACHE[CAP] = _build(CAP)
    return _CACHE[CAP]


def _prep_in_maps(inputs, CAP):
    f32 = lambda a: np.ascontiguousarray(np.asarray(a), dtype=np.float32)
    hidden = f32(inputs["hidden"]); mu_prev = f32(inputs["mu_prev"]); velocity = f32(inputs["velocity"])
    positions = np.asarray(inputs["positions"]).astype(np.float32)
    token_ids = np.asarray(inputs["token_ids"])
    inv_freq = THETA ** (-np.arange(0, DH, 2, dtype=np.float32) / DH)
    ang = positions[:, None] * inv_freq
    cs = np.concatenate([np.cos(ang), np.sin(ang)], axis=1).astype(np.float32)  # [N, 64]
    base_ids = (token_ids % E).astype(np.int64)
    boh = (np.eye(E, dtype=np.float32)[base_ids] * BASE_SCALE).astype(np.float32)
    cowx = np.concatenate([f32(inputs["ctrl_out_w"]), f32(inputs["ctrl_out_b"])[None, :]], axis=0)
    shared = dict(
        wq=f32(inputs["wq"]), wmq=f32(inputs["w_mu_q"]),
        wk=f32(inputs["wk"]), wmk=f32(inputs["w_mu_k"]),
        wv=f32(inputs["wv"]), wmv=f32(inputs["w_mu_v"]),
        wo=f32(inputs["wo"]), dynw=f32(inputs["dyn_mu_proj_w"]),
        ciw=f32(inputs["ctrl_in_w"]), cib=f32(inputs["ctrl_in_b"])[None, :],
        cowx=cowx, mrw=f32(inputs["mu_router_w"]),
        ln1=f32(inputs["ln1_w"])[None, :], ln2=f32(inputs["ln2_w"])[None, :],
        qnw=np.tile(f32(inputs["qnorm_w"]), H)[None, :],
        knw=np.tile(f32(inputs["knorm_w"]), KV)[None, :],
        dmu=f32(inputs["dyn_mu"])[None, :],
        trib=np.triu(np.ones((P, P), np.float32), 1),
        ident=np.eye(P, dtype=np.float32),
        iotac=np.arange(CAP, dtype=np.float32)[None, :],
        iota8=np.arange(E, dtype=np.float32)[None, :],
    )
    wg = f32(inputs["w_gate"]); wu = f32(inputs["w_up"]); wd = f32(inputs["w_down"])
    in_maps = []
    for c in range(NC_):
        sl = slice(c * NT, (c + 1) * NT)
        m = dict(shared)
        m.update(
            hid=hidden[sl], mu=mu_prev[sl], vel=velocity[sl],
            cs=cs[sl], boh=boh[sl],
            wg=np.ascontiguousarray(wg[c]), wu=np.ascontiguousarray(wu[c]),
            wd=np.ascontiguousarray(wd[c]),
            mye=np.full((P, 1), float(c), np.float32),
        )
        in_maps.append(m)
    return in_maps, base_ids


def _pick_cap(base_ids):
    counts = np.bincount(base_ids, minlength=E)
    cap = int(np.ceil((counts.max() + 96) / P) * P)
    return max(256, min(512, cap))


def kernel(**inputs):
    token_ids = np.asarray(inputs["token_ids"])
    base_ids = (token_ids % E).astype(np.int64)
    CAP = _pick_cap(base_ids)
    nc = _get_nc(CAP)
    in_maps, _ = _prep_in_maps(inputs, CAP)
    res = run_bass_kernel_spmd(nc, in_maps, core_ids=list(range(NC_)))
    hidden = np.concatenate([res.results[c]["oh"] for c in range(NC_)], axis=0)
    v_next = np.concatenate([res.results[c]["ov"] for c in range(NC_)], axis=0)
    mu_cur = np.concatenate([res.results[c]["om"] for c in range(NC_)], axis=0)
    # capacity sanity check (routing is dominated by the base one-hot; CAP has
    # ~100-token margin, so this should never fire for uniform token_ids)
    mrw = np.asarray(inputs["mu_router_w"], dtype=np.float32)
    logits = mu_cur @ mrw + np.eye(E, dtype=np.float32)[base_ids] * BASE_SCALE
    eids = logits.argmax(-1)
    assert np.bincount(eids, minlength=E).max() <= CAP, "expert capacity overflow"
    return hidden, v_next, mu_cur


# revision 17
# speedup vs baseline: 1.0102x; 1.0102x over previous
"""Trainium2 Bass kernel for nn_ComplexityDecoderLayer (moe_routing).

Strategy (8 NeuronCores, SPMD):
  - Token-parallel attention + PID dynamics: each core owns 256 of 2048 tokens.
    K/V are computed per-shard (qk-norm + RoPE) and AllGathered in two
    head-pair chunks so attention on the first pair overlaps the second
    chunk's transfer.
  - Attention uses exp without max-subtraction (scores are O(1) after qk-norm)
    in transposed layout [keys, q]; softmax numerator and denominator come out
    of one PSUM accumulation via a ones-column appended to V.
  - Expert-parallel MoE with AllToAll token dispatch: each core sorts its own
    256 tokens by destination expert into an [8 x 64, 1024] send buffer using
    0/1 permutation matmuls built from a triangular-matmul prefix sum, then a
    2-chunk AllToAll delivers each expert its tokens; the expert FFN runs on
    the 512 received rows; results return via a second (chunked) AllToAll and
    are unsorted locally. No ReduceScatter needed.
All heavy compute is fp32 on device; the host only slices/concats inputs,
precomputes RoPE cos/sin tables from `positions` and the base-id one-hot from
`token_ids`, and reassembles the three outputs.
"""

import numpy as np

import concourse.mybir as mybir
import concourse.tile as tile
from concourse import bacc
from concourse.bass_utils import run_bass_kernel_spmd

F32 = mybir.dt.float32
F32R = mybir.dt.float32r
AF = mybir.ActivationFunctionType
OP = mybir.AluOpType
AX = mybir.AxisListType

P = 128
N, D, H, KV, DH, E, FF, CH = 2048, 1024, 16, 4, 64, 8, 2048, 64
NC_ = 8
NT = N // NC_          # 256 tokens per core
RT = NT // P           # 2 row tiles
DT_ = D // P           # 8
FT = FF // P           # 16
JT = N // P            # 16 global token tiles
C2 = 64                # per (src, dst) expert-dispatch capacity
SR = E * C2            # 512 rows through each expert
SRT = SR // P          # 4
EPS = 1e-6
THETA = 10000.0
DTC = 0.1
BASE_SCALE = 10.0

_CACHE = {}


def _build():
    nc = bacc.Bacc(target_bir_lowering=False)

    def par(name, shp):
        return nc.declare_dram_parameter(name, list(shp), F32, isOutput=False)

    hid_p = par("hid", [NT, D])
    mu_p = par("mu", [NT, D])
    vel_p = par("vel", [NT, D])
    cs_p = par("cs", [NT, 2 * 32])          # [cos | sin]
    boh_p = par("boh", [NT, E])             # BASE_SCALE * one_hot(token_ids % E)
    wq_p = par("wq", [D, D])
    wmq_p = par("wmq", [D, D])
    wk_p = par("wk", [D, KV * DH])
    wmk_p = par("wmk", [D, KV * DH])
    wv_p = par("wv", [D, KV * DH])
    wmv_p = par("wmv", [D, KV * DH])
    wo_p = par("wo", [D, D])
    dynw_p = par("dynw", [D, D])
    ciw_p = par("ciw", [2 * D, CH])
    cib_p = par("cib", [1, CH])
    cowx_p = par("cowx", [CH + 1, 3 * D])   # [ctrl_out_w ; ctrl_out_b]
    mrw_p = par("mrw", [D, E])
    wg_p = par("wg", [D, FF])               # this core's expert
    wu_p = par("wu", [D, FF])
    wd_p = par("wd", [FF, D])
    ln1_p = par("ln1", [1, D])
    ln2_p = par("ln2", [1, D])
    qnw_p = par("qnw", [1, D])              # qnorm_w tiled 16x
    knw_p = par("knw", [1, KV * DH])        # knorm_w tiled 4x
    dmu_p = par("dmu", [1, D])
    trib_p = par("trib", [P, P])            # strict upper triangular ones
    ident_p = par("ident", [P, P])
    iotac_p = par("iotac", [1, C2])
    iota8_p = par("iota8", [1, E])

    oh_p = nc.declare_dram_parameter("oh", [NT, D], F32, isOutput=True)
    ov_p = nc.declare_dram_parameter("ov", [NT, D], F32, isOutput=True)
    om_p = nc.declare_dram_parameter("om", [NT, D], F32, isOutput=True)

    with tile.TileContext(nc) as tc:
        from contextlib import ExitStack
        with ExitStack() as TOP:
            dram = TOP.enter_context(tc.tile_pool(name="dram", bufs=1, space="DRAM"))
            const = TOP.enter_context(tc.tile_pool(name="const", bufs=1))
            ps = TOP.enter_context(tc.tile_pool(name="ps", bufs=1, space="PSUM"))
            ws = TOP.enter_context(tc.tile_pool(name="wstream", bufs=1))
            work = TOP.enter_context(tc.tile_pool(name="work", bufs=1))
            top = TOP.enter_context(tc.tile_pool(name="top", bufs=1))

            # phase-scoped pools (manually closed LIFO per side to free SBUF)
            cm_dyn = tc.tile_pool(name="p_dyn", bufs=1); p_dyn = cm_dyn.__enter__()
            cm_att = tc.tile_pool(name="p_att", bufs=1); p_att = cm_att.__enter__()
            cm_hm = tc.tile_pool(name="p_hm", bufs=1); p_hm = cm_hm.__enter__()

            # ---------------- constants ----------------
            ident = const.tile([P, P], F32, name="identc")
            nc.sync.dma_start(out=ident[:, :], in_=ident_p[:, :])
            ident_r = const.tile([P, P], F32R, name="identr")
            nc.gpsimd.dma_start(out=ident_r[:, :], in_=ident_p[:, :])
            trib = const.tile([P, P], F32, name="tribc")
            nc.sync.dma_start(out=trib[:, :], in_=trib_p[:, :])
            iota64b = const.tile([P, C2], F32, name="iota64b")
            nc.sync.dma_start(out=iota64b[:, :], in_=iotac_p[:, :].to_broadcast((P, C2)))
            iota8b = const.tile([P, E], F32, name="iota8b")
            nc.sync.dma_start(out=iota8b[:, :], in_=iota8_p[:, :].to_broadcast((P, E)))
            epsb = const.tile([P, 1], F32, name="epsb")
            nc.vector.memset(epsb[:, :], EPS)
            ones_r = const.tile([1, NT], F32, name="onesr")   # lhsT row for bias matmuls
            nc.vector.memset(ones_r[:, :], 1.0)
            ones_c = const.tile([P, 1], F32, name="onesc")    # rhs col for colsum
            nc.vector.memset(ones_c[:, :], 1.0)
            dmu_sb = const.tile([1, D], F32, name="dmusb")
            nc.sync.dma_start(out=dmu_sb[:, :], in_=dmu_p[:, :])
            cib_sb = const.tile([1, CH], F32, name="cibsb")
            nc.sync.dma_start(out=cib_sb[:, :], in_=cib_p[:, :])

            ln1b = p_dyn.tile([P, D], F32, name="ln1b")
            nc.sync.dma_start(out=ln1b[:, :], in_=ln1_p[:, :].to_broadcast((P, D)))
            ln2b = p_dyn.tile([P, D], F32, name="ln2b")
            nc.sync.dma_start(out=ln2b[:, :], in_=ln2_p[:, :].to_broadcast((P, D)))
            qnwb = p_dyn.tile([P, D], F32, name="qnwb")
            nc.sync.dma_start(out=qnwb[:, :], in_=qnw_p[:, :].to_broadcast((P, D)))
            knwb = p_dyn.tile([P, KV * DH], F32, name="knwb")
            nc.sync.dma_start(out=knwb[:, :], in_=knw_p[:, :].to_broadcast((P, KV * DH)))
            ciw_sb = p_dyn.tile([P, 16 * CH], F32, name="ciwsb")  # [2048,64] -> [128, 16*64]
            nc.sync.dma_start(
                out=ciw_sb[:, :].rearrange("p (j c) -> p j c", j=16),
                in_=ciw_p[:, :].rearrange("(j p) c -> p j c", p=P),
            )
            mrw_sb = p_dyn.tile([P, DT_ * E], F32, name="mrwsb")  # [1024,8] -> [128, 8*8]
            nc.sync.dma_start(
                out=mrw_sb[:, :].rearrange("p (j c) -> p j c", j=DT_),
                in_=mrw_p[:, :].rearrange("(j p) c -> p j c", p=P),
            )
            cos_sb = [p_dyn.tile([P, 32], F32, name=f"cos{rt}") for rt in range(RT)]
            sin_sb = [p_dyn.tile([P, 32], F32, name=f"sin{rt}") for rt in range(RT)]
            for rt in range(RT):
                nc.sync.dma_start(out=cos_sb[rt][:, :], in_=cs_p[rt * P:(rt + 1) * P, 0:32])
                nc.sync.dma_start(out=sin_sb[rt][:, :], in_=cs_p[rt * P:(rt + 1) * P, 32:64])

            # ---------------- DRAM internals ----------------
            # kv chunk i holds kv-head pair (2i, 2i+1): [k_pair | v_pair]
            kv_in = [dram.tile([NT, 2 * P], F32, name=f"kvin{i}") for i in range(2)]
            kv_full = [dram.tile([N, 2 * P], F32, name=f"kvfull{i}", addr_space="Shared") for i in range(2)]
            a2a_in = [dram.tile([SR, 512], F32, name=f"a2ain{i}") for i in range(2)]
            a2a_out = [dram.tile([SR, 512], F32, name=f"a2aout{i}") for i in range(2)]
            bk_in = [dram.tile([SR, 512], F32, name=f"bkin{i}") for i in range(2)]
            bk_out = [dram.tile([SR, 512], F32, name=f"bkout{i}") for i in range(2)]

            def peT(src_ap, dst_ap, engine, idt=None):
                """dst = src^T via PE transpose (src [p, f] -> dst [f, p])."""
                if idt is None:
                    idt = ident
                f = src_ap.shape[-1]
                p_ = src_ap.shape[0]
                pt = ps.tile([P, P], src_ap.dtype, tag="pt", bufs=2, name="pt")
                nc.tensor.transpose(pt[0:f, 0:p_], src_ap, idt[0:p_, 0:p_])
                engine(dst_ap, pt[0:f, 0:p_])

            vcopy = nc.vector.tensor_copy
            scopy = nc.scalar.copy

            def rmsnorm(dst, src, wb, ddim):
                t = work.tile([P, ddim], F32, tag="wk1024", bufs=4, name="rmst")
                sS = work.tile([P, 1], F32, tag="rms_s", bufs=4, name="rmss")
                nc.scalar.activation(t[:, 0:ddim], src, AF.Square, accum_out=sS[:, :])
                sq = work.tile([P, 1], F32, tag="rms_q", bufs=4, name="rmsq")
                nc.scalar.activation(sq[:, :], sS[:, :], AF.Sqrt, bias=epsb[:, :], scale=1.0 / ddim)
                rs_ = work.tile([P, 1], F32, tag="rms_r", bufs=4, name="rmsr")
                nc.vector.reciprocal(rs_[:, :], sq[:, :])
                nc.vector.tensor_scalar_mul(dst, src, rs_[:, :])
                nc.vector.tensor_tensor(dst, dst, wb, OP.mult)

            def headnorm(qr, nh, wb):
                for hh in range(nh):
                    sl = qr[:, hh * DH:(hh + 1) * DH]
                    t = work.tile([P, DH], F32, tag="hn_t", bufs=2, name="hnt")
                    sS = work.tile([P, 1], F32, tag="hn_s", bufs=4, name="hns")
                    nc.scalar.activation(t[:, :], sl, AF.Square, accum_out=sS[:, :])
                    sq = work.tile([P, 1], F32, tag="hn_q", bufs=4, name="hnq")
                    nc.scalar.activation(sq[:, :], sS[:, :], AF.Sqrt, bias=epsb[:, :], scale=1.0 / DH)
                    rs_ = work.tile([P, 1], F32, tag="hn_r", bufs=4, name="hnr")
                    nc.vector.reciprocal(rs_[:, :], sq[:, :])
                    nc.vector.tensor_scalar_mul(sl, sl, rs_[:, :])
                nc.vector.tensor_tensor(qr, qr, wb[:, 0:qr.shape[-1]], OP.mult)

            def rope(dst, src, rt, nh):
                s3 = src.rearrange("p (h d) -> p h d", h=nh)
                d3 = dst.rearrange("p (h d) -> p h d", h=nh)
                c3 = cos_sb[rt][:, :].rearrange("p (o d) -> p o d", o=1).to_broadcast((P, nh, 32))
                n3 = sin_sb[rt][:, :].rearrange("p (o d) -> p o d", o=1).to_broadcast((P, nh, 32))
                tmp = work.tile([P, H * 32], F32, tag="rope_t", bufs=2, name="ropet")
                t3 = tmp[:, 0:nh * 32].rearrange("p (h d) -> p h d", h=nh)
                x1 = s3[:, :, 0:32]
                x2 = s3[:, :, 32:64]
                nc.vector.tensor_tensor(d3[:, :, 0:32], x1, c3, OP.mult)
                nc.vector.tensor_tensor(t3, x2, n3, OP.mult)
                nc.vector.tensor_tensor(d3[:, :, 0:32], d3[:, :, 0:32], t3, OP.subtract)
                nc.vector.tensor_tensor(d3[:, :, 32:64], x2, c3, OP.mult)
                nc.vector.tensor_tensor(t3, x1, n3, OP.mult)
                nc.vector.tensor_tensor(d3[:, :, 32:64], d3[:, :, 32:64], t3, OP.add)

            # ================= Phase 1: h/mu transposes, k/v first =================
            hid = [p_dyn.tile([P, D], F32, name=f"hid{rt}") for rt in range(RT)]
            vel = [p_dyn.tile([P, D], F32, name=f"vel{rt}") for rt in range(RT)]
            velT = [p_dyn.tile([P, NT], F32, name=f"velT{k}") for k in range(DT_)]
            hT = [p_hm.tile([P, NT], F32, name=f"hT{k}") for k in range(DT_)]
            muT = [p_hm.tile([P, NT], F32, name=f"muT{k}") for k in range(DT_)]
            qrows = [p_hm.tile([P, D], F32, name=f"qrows{rt}") for rt in range(RT)]
            h2 = [top.tile([P, D], F32, name=f"h2{rt}") for rt in range(RT)]
            xr = [top.tile([P, D], F32R, name=f"xr{rt}") for rt in range(RT)]
            eid_loc = top.tile([P, RT], F32, name="eidloc")

            for rt in range(RT):
                nc.sync.dma_start(out=hid[rt][:, :], in_=hid_p[rt * P:(rt + 1) * P, :])
                h = work.tile([P, D], F32, tag="wk1024", bufs=4, name="hrows")
                rmsnorm(h[:, :], hid[rt][:, :], ln1b[:, :], D)
                mrow = work.tile([P, D], F32, tag="wk1024", bufs=4, name="murows")
                nc.sync.dma_start(out=mrow[:, :], in_=mu_p[rt * P:(rt + 1) * P, :])
                for k in range(DT_):
                    peT(h[:, k * P:(k + 1) * P], hT[k][:, rt * P:(rt + 1) * P], vcopy)
                    peT(mrow[:, k * P:(k + 1) * P], muT[k][:, rt * P:(rt + 1) * P], vcopy)

            # k/v rows first so the kv AllGathers overlap the q-side work
            for rt in range(RT):
                pk = ps.tile([P, KV * DH], F32, tag="big", bufs=4, name="pk")
                pv = ps.tile([P, KV * DH], F32, tag="big", bufs=4, name="pv")
                i = 0
                for lhsT, wp1, wp2 in ((hT, wk_p, wv_p), (muT, wmk_p, wmv_p)):
                    for k in range(DT_):
                        wt1 = ws.tile([P, KV * DH], F32, tag="w256", bufs=4, name="wt1")
                        nc.sync.dma_start(out=wt1[:, :], in_=wp1[k * P:(k + 1) * P, :])
                        wt2 = ws.tile([P, KV * DH], F32, tag="w256", bufs=4, name="wt2")
                        nc.sync.dma_start(out=wt2[:, :], in_=wp2[k * P:(k + 1) * P, :])
                        nc.tensor.matmul(pk[:, :], lhsT[k][:, rt * P:(rt + 1) * P],
                                         wt1[:, :], start=(i == 0), stop=(i == 2 * DT_ - 1))
                        nc.tensor.matmul(pv[:, :], lhsT[k][:, rt * P:(rt + 1) * P],
                                         wt2[:, :], start=(i == 0), stop=(i == 2 * DT_ - 1))
                        i += 1
                krow = p_hm.tile([P, KV * DH], F32, tag="kv256", bufs=3, name="krow")
                vrow = p_hm.tile([P, KV * DH], F32, tag="kv256b", bufs=3, name="vrow")
                vcopy(krow[:, :], pk[:, :])
                vcopy(vrow[:, :], pv[:, :])
                headnorm(krow[:, :], KV, knwb)
                rk = p_hm.tile([P, KV * DH], F32, tag="kv256c", bufs=3, name="rk")
                rope(rk[:, :], krow[:, :], rt, KV)
                for i in range(2):
                    nc.sync.dma_start(out=kv_in[i][rt * P:(rt + 1) * P, 0:P], in_=rk[:, i * P:(i + 1) * P])
                    nc.sync.dma_start(out=kv_in[i][rt * P:(rt + 1) * P, P:2 * P], in_=vrow[:, i * P:(i + 1) * P])

            for i in range(2):
                nc.gpsimd.collective_compute(
                    "AllGather", OP.bypass, replica_groups=[list(range(NC_))],
                    ins=[kv_in[i][:, :].opt()], outs=[kv_full[i][:, :].opt()],
                )

            # q rows = h @ wq + mu @ wmq (overlaps kv AllGathers)
            for nt in range(2):
                pq = [ps.tile([P, 512], F32, tag="big", bufs=4, name="pq") for _ in range(RT)]
                i = 0
                for lhsT, w_p in ((hT, wq_p), (muT, wmq_p)):
                    for k in range(DT_):
                        wt = ws.tile([P, 512], F32, tag="w512", bufs=3, name="wt")
                        nc.sync.dma_start(out=wt[:, :], in_=w_p[k * P:(k + 1) * P, nt * 512:(nt + 1) * 512])
                        for rt in range(RT):
                            nc.tensor.matmul(pq[rt][:, :], lhsT[k][:, rt * P:(rt + 1) * P], wt[:, :],
                                             start=(i == 0), stop=(i == 2 * DT_ - 1))
                        i += 1
                for rt in range(RT):
                    vcopy(qrows[rt][:, nt * 512:(nt + 1) * 512], pq[rt][:, :])

            qT = [p_att.tile([DH, NT], F32, name=f"qT{hh}") for hh in range(H)]
            for rt in range(RT):
                headnorm(qrows[rt][:, :], H, qnwb)
                rq = work.tile([P, D], F32, tag="wk1024", bufs=4, name="rq")
                rope(rq[:, :], qrows[rt][:, :], rt, H)
                for k in range(DT_):
                    pt = ps.tile([P, P], F32, tag="pt", bufs=2, name="ptq")
                    nc.tensor.transpose(pt[:, :], rq[:, k * P:(k + 1) * P], ident[:, :])
                    vcopy(qT[2 * k][:, rt * P:(rt + 1) * P], pt[0:DH, :])
                    vcopy(qT[2 * k + 1][:, rt * P:(rt + 1) * P], pt[DH:P, :])

            cm_hm.__exit__(None, None, None)  # free hT/muT/qrows

            # ================= Phase 2+3: unpack k/v per chunk; attention =================
            cm_o = tc.tile_pool(name="p_o", bufs=1, side="right"); p_o = cm_o.__enter__()
            oT = [p_o.tile([P, NT], F32, name=f"oT{k}") for k in range(DT_)]
            kT = [p_att.tile([DH, N], F32, name=f"kT{g}") for g in range(KV)]
            vext4 = [p_att.tile([P, JT * 65], F32, name=f"vext{g}") for g in range(KV)]
            for chunk in range(2):
                for g in (2 * chunk, 2 * chunk + 1):
                    nc.vector.memset(vext4[g][:, :], 1.0)
                for tt in range(JT):
                    kl = p_att.tile([P, P], F32, tag="kl", bufs=2, name="kl")
                    nc.sync.dma_start(out=kl[:, :], in_=kv_full[chunk][tt * P:(tt + 1) * P, 0:P])
                    pt = ps.tile([P, P], F32, tag="pt", bufs=2, name="ptk")
                    nc.tensor.transpose(pt[:, :], kl[:, :], ident[:, :])
                    vcopy(kT[2 * chunk][:, tt * P:(tt + 1) * P], pt[0:DH, :])
                    vcopy(kT[2 * chunk + 1][:, tt * P:(tt + 1) * P], pt[DH:P, :])
                    for gl in range(2):
                        g = 2 * chunk + gl
                        nc.sync.dma_start(out=vext4[g][:, tt * 65:tt * 65 + 64],
                                          in_=kv_full[chunk][tt * P:(tt + 1) * P, P + gl * DH:P + (gl + 1) * DH])
                for g in (2 * chunk, 2 * chunk + 1):
                    vext = vext4[g]
                    for hq in range(H // KV):
                        hh = g * (H // KV) + hq
                        qTh = qT[hh][:, :]
                        pO = ps.tile([65, NT], F32, tag="oext", bufs=2, name="pO")
                        for tt in range(JT):
                            pS = ps.tile([P, NT], F32, tag="big", bufs=4, name="pS")
                            nc.tensor.matmul(pS[:, :], kT[g][:, tt * P:(tt + 1) * P],
                                             qTh, start=True, stop=True)
                            ex = p_att.tile([P, NT], F32, tag="ex", bufs=3, name="ex")
                            nc.scalar.activation(ex[:, :], pS[:, :], AF.Exp, scale=0.125)
                            nc.tensor.matmul(pO[:, :], vext[:, tt * 65:(tt + 1) * 65], ex[:, :],
                                             start=(tt == 0), stop=(tt == JT - 1))
                        rd = p_att.tile([1, NT], F32, tag="rd", bufs=2, name="rd")
                        nc.vector.reciprocal(rd[:, :], pO[64:65, :])
                        dbn = dram.tile([1, NT], F32, tag="dbn", bufs=2, name="dbn")
                        nc.sync.dma_start(out=dbn[:, :], in_=rd[:, :])
                        rdb = p_att.tile([DH, NT], F32, tag="rdb", bufs=2, name="rdb")
                        nc.sync.dma_start(out=rdb[:, :], in_=dbn[:, :].to_broadcast((DH, NT)))
                        nc.vector.tensor_tensor(oT[hh // 2][(hh % 2) * DH:(hh % 2 + 1) * DH, :],
                                                pO[0:DH, :], rdb[:, :], OP.mult)

            cm_att.__exit__(None, None, None)  # free qT/kT/vext

            # ================= Phase 4: wo + dynamics + router =================
            cm_wo = tc.tile_pool(name="p_wo", bufs=1); p_wo = cm_wo.__enter__()
            orows = [p_wo.tile([P, D], F32, name=f"orows{rt}") for rt in range(RT)]
            for nt in range(2):
                po = [ps.tile([P, 512], F32, tag="big", bufs=4, name="po") for _ in range(RT)]
                for k in range(DT_):
                    wt = ws.tile([P, 512], F32, tag="w512", bufs=3, name="wot")
                    nc.sync.dma_start(out=wt[:, :], in_=wo_p[k * P:(k + 1) * P, nt * 512:(nt + 1) * 512])
                    for rt in range(RT):
                        nc.tensor.matmul(po[rt][:, :], oT[k][:, rt * P:(rt + 1) * P], wt[:, :],
                                         start=(k == 0), stop=(k == DT_ - 1))
                for rt in range(RT):
                    vcopy(orows[rt][:, nt * 512:(nt + 1) * 512], po[rt][:, :])

            oTw = [p_wo.tile([P, NT], F32, name=f"oTw{k}") for k in range(DT_)]
            for rt in range(RT):
                for k in range(DT_):
                    peT(orows[rt][:, k * P:(k + 1) * P], oTw[k][:, rt * P:(rt + 1) * P], vcopy)
            cm_o.__exit__(None, None, None)  # free oT

            # mu_cur = dyn_mu + o @ dynw
            mucur = [p_wo.tile([P, D], F32, name=f"mucur{rt}") for rt in range(RT)]
            for nt in range(2):
                pm = [ps.tile([P, 512], F32, tag="big", bufs=4, name="pm") for _ in range(RT)]
                for k in range(DT_):
                    wt = ws.tile([P, 512], F32, tag="w512", bufs=3, name="dynt")
                    nc.sync.dma_start(out=wt[:, :], in_=dynw_p[k * P:(k + 1) * P, nt * 512:(nt + 1) * 512])
                    for rt in range(RT):
                        nc.tensor.matmul(pm[rt][:, :], oTw[k][:, rt * P:(rt + 1) * P], wt[:, :],
                                         start=(k == 0), stop=False)
                for rt in range(RT):
                    nc.tensor.matmul(pm[rt][:, :], ones_r[0:1, rt * P:(rt + 1) * P],
                                     dmu_sb[0:1, nt * 512:(nt + 1) * 512], start=False, stop=True)
                    vcopy(mucur[rt][:, nt * 512:(nt + 1) * 512], pm[rt][:, :])
            for rt in range(RT):
                nc.sync.dma_start(out=om_p[rt * P:(rt + 1) * P, :], in_=mucur[rt][:, :])

            # router early: eid depends only on mu_cur
            mcT = [p_wo.tile([P, NT], F32, name=f"mcT{k}") for k in range(DT_)]
            for rt in range(RT):
                for k in range(DT_):
                    peT(mucur[rt][:, k * P:(k + 1) * P], mcT[k][:, rt * P:(rt + 1) * P], vcopy)
            for rt in range(RT):
                pr = ps.tile([P, E], F32, tag="big", bufs=4, name="pr")
                for k in range(DT_):
                    nc.tensor.matmul(pr[:, :], mcT[k][:, rt * P:(rt + 1) * P],
                                     mrw_sb[:, k * E:(k + 1) * E], start=(k == 0), stop=(k == DT_ - 1))
                cmb = work.tile([P, E], F32, tag="cmb", bufs=2, name="cmb")
                bohs = work.tile([P, E], F32, tag="bohs", bufs=2, name="bohs")
                nc.sync.dma_start(out=bohs[:, :], in_=boh_p[rt * P:(rt + 1) * P, :])
                nc.vector.tensor_tensor(cmb[:, :], pr[:, :], bohs[:, :], OP.add)
                mx = work.tile([P, 1], F32, tag="mx", bufs=2, name="mx")
                nc.vector.reduce_max(mx[:, :], cmb[:, :], axis=AX.X)
                nc.vector.tensor_scalar(cmb[:, :], cmb[:, :], mx[:, :], None, OP.is_equal)
                nc.vector.tensor_tensor(cmb[:, :], cmb[:, :], iota8b[:, :], OP.mult)
                nc.vector.reduce_sum(eid_loc[:, rt:rt + 1], cmb[:, :], axis=AX.X)

            # ctrl MLP (velocity loaded/transposed here, off the critical path)
            for rt in range(RT):
                nc.sync.dma_start(out=vel[rt][:, :], in_=vel_p[rt * P:(rt + 1) * P, :])
            for rt in range(RT):
                for k in range(DT_):
                    peT(vel[rt][:, k * P:(k + 1) * P], velT[k][:, rt * P:(rt + 1) * P], vcopy)
            ctT = p_wo.tile([CH + 1, NT], F32, name="ctT")
            nc.vector.memset(ctT[CH:CH + 1, :], 1.0)
            for rt in range(RT):
                pc = ps.tile([P, CH], F32, tag="big", bufs=4, name="pc")
                for k in range(DT_):
                    nc.tensor.matmul(pc[:, :], oTw[k][:, rt * P:(rt + 1) * P],
                                     ciw_sb[:, k * CH:(k + 1) * CH], start=(k == 0), stop=False)
                for k in range(DT_):
                    nc.tensor.matmul(pc[:, :], velT[k][:, rt * P:(rt + 1) * P],
                                     ciw_sb[:, (DT_ + k) * CH:(DT_ + k + 1) * CH], start=False, stop=False)
                nc.tensor.matmul(pc[:, :], ones_r[0:1, rt * P:(rt + 1) * P], cib_sb[0:1, :],
                                 start=False, stop=True)
                ct = work.tile([P, CH], F32, tag="ct", bufs=2, name="ct")
                nc.scalar.activation(ct[:, :], pc[:, :], AF.Silu)
                peT(ct[:, :], ctT[0:CH, rt * P:(rt + 1) * P], vcopy)

            abg = [[p_wo.tile([P, D], F32, name=f"abg{i}{rt}") for rt in range(RT)] for i in range(3)]
            for nt in (0, 1, 4, 5, 2, 3):
                cw = ws.tile([CH + 1, 512], F32, tag="cow", bufs=2, name="cw")
                nc.sync.dma_start(out=cw[:, :], in_=cowx_p[:, nt * 512:(nt + 1) * 512])
                for rt in range(RT):
                    pb = ps.tile([P, 512], F32, tag="big", bufs=4, name="pb")
                    nc.tensor.matmul(pb[:, :], ctT[:, rt * P:(rt + 1) * P], cw[:, :],
                                     start=True, stop=True)
                    dst = abg[nt // 2][rt][:, (nt % 2) * 512:(nt % 2 + 1) * 512]
                    if nt // 2 != 1:
                        nc.scalar.activation(dst, pb[:, :], AF.Sigmoid)
                    else:
                        # softplus = ln(1 + exp(x)); Exp/Ln share one ACT table.
                        # exp overflow -> inf -> ln -> inf -> min(.,2) still correct.
                        nc.scalar.activation(dst, pb[:, :], AF.Exp)
                        nc.vector.tensor_scalar_add(dst, dst, 1.0)
                        nc.scalar.activation(dst, dst, AF.Ln)
            for rt in range(RT):
                nc.vector.tensor_scalar_min(abg[1][rt][:, :], abg[1][rt][:, :], 2.0)

            # dynamics elementwise + x
            for rt in range(RT):
                err = work.tile([P, D], F32, tag="wk1024", bufs=4, name="err")
                nc.vector.tensor_tensor(err[:, :], orows[rt][:, :], mucur[rt][:, :], OP.subtract)
                av = work.tile([P, D], F32, tag="wk1024", bufs=4, name="av")
                nc.vector.tensor_tensor(av[:, :], abg[0][rt][:, :], vel[rt][:, :], OP.mult)
                nc.vector.tensor_tensor(err[:, :], abg[1][rt][:, :], err[:, :], OP.mult)
                nc.vector.tensor_tensor(av[:, :], av[:, :], err[:, :], OP.subtract)
                nc.vector.tensor_scalar_min(av[:, :], av[:, :], 10.0)
                nc.vector.tensor_scalar_max(av[:, :], av[:, :], -10.0)
                nc.sync.dma_start(out=ov_p[rt * P:(rt + 1) * P, :], in_=av[:, :])
                gv = work.tile([P, D], F32, tag="wk1024", bufs=4, name="gv")
                nc.vector.tensor_tensor(gv[:, :], abg[2][rt][:, :], av[:, :], OP.mult)
                nc.vector.tensor_scalar_mul(gv[:, :], gv[:, :], DTC)
                nc.vector.tensor_tensor(gv[:, :], gv[:, :], orows[rt][:, :], OP.add)
                nc.vector.tensor_tensor(h2[rt][:, :], gv[:, :], hid[rt][:, :], OP.add)
                rmsnorm(xr[rt][:, :], h2[rt][:, :], ln2b[:, :], D)

            cm_wo.__exit__(None, None, None)   # free orows/oTw/mucur/ctT/abg/mcT
            cm_dyn.__exit__(None, None, None)  # free hid/vel/velT/phase consts

            # ================= Phase 5: local sort + AllToAll MoE =================
            cm_moe = tc.tile_pool(name="p_moe", bufs=1); p_moe = cm_moe.__enter__()
            # PT_send [256 tok, 8*C2]: PT[t, d*C2+s] = 1 iff token t is the s-th
            # token (in order) among this core's tokens routed to expert d
            PTs = [p_moe.tile([P, SR], F32R, name=f"PTs{j}") for j in range(RT)]
            for d in range(E):
                maskd = p_moe.tile([P, RT], F32, tag="maskd", bufs=2, name="maskd")
                nc.vector.tensor_scalar(maskd[:, :], eid_loc[:, :], float(d), None, OP.is_equal)
                pexl = ps.tile([P, RT], F32, tag="pt", bufs=2, name="pexl")
                nc.tensor.matmul(pexl[:, :], trib[:, :], maskd[:, :], start=True, stop=True)
                pcs2 = ps.tile([RT, 1], F32, tag="pt", bufs=2, name="pcs2")
                nc.tensor.matmul(pcs2[:, :], maskd[:, :], ones_c[:, :], start=True, stop=True)
                cs2 = p_moe.tile([RT, 1], F32, tag="cs2", bufs=2, name="cs2")
                vcopy(cs2[:, :], pcs2[:, :])
                csb0 = p_moe.tile([P, 1], F32, tag="csb0", bufs=2, name="csb0")
                nc.gpsimd.partition_broadcast(csb0[:, :], cs2[0:1, 0:1])
                rankd = p_moe.tile([P, RT], F32, tag="rankd", bufs=2, name="rankd")
                vcopy(rankd[:, 0:1], pexl[:, 0:1])
                nc.vector.tensor_tensor(rankd[:, 1:2], pexl[:, 1:2], csb0[:, :], OP.add)
                for j in range(RT):
                    nc.vector.tensor_scalar(PTs[j][:, d * C2:(d + 1) * C2], iota64b[:, :],
                                            rankd[:, j:j + 1], maskd[:, j:j + 1],
                                            OP.is_equal, OP.mult)
            # x_send = PT_send^T @ x_rows -> [SR, 1024], sent as 2 column chunks
            for half in range(2):
                for sm in range(SRT):
                    pxs = ps.tile([P, 512], F32, tag="big", bufs=4, name="pxs")
                    for j in range(RT):
                        nc.tensor.matmul(pxs[:, :], PTs[j][:, sm * P:(sm + 1) * P],
                                         xr[j][:, half * 512:(half + 1) * 512],
                                         start=(j == 0), stop=(j == RT - 1))
                    xs = p_moe.tile([P, 512], F32, tag="xsend", bufs=3, name="xs")
                    scopy(xs[:, :], pxs[:, :])
                    nc.sync.dma_start(out=a2a_in[half][sm * P:(sm + 1) * P, :], in_=xs[:, :])
                nc.gpsimd.collective_compute(
                    "AllToAll", OP.bypass, replica_groups=[list(range(NC_))],
                    ins=[a2a_in[half][:, :].opt()], outs=[a2a_out[half][:, :].opt()],
                )
            # PT_send^T for the un-sort at the end
            PTT = [p_moe.tile([P, NT], F32R, name=f"PTT{sm}") for sm in range(SRT)]
            for j in range(RT):
                for sm in range(SRT):
                    peT(PTs[j][:, sm * P:(sm + 1) * P], PTT[sm][:, j * P:(j + 1) * P], scopy, idt=ident_r)

            # received tokens -> transposed activations xsT [1024, SR]
            xsT = [p_moe.tile([P, SR], F32R, name=f"xsT{k}") for k in range(DT_)]
            for half in range(2):
                for sm in range(SRT):
                    xrc = p_moe.tile([P, 512], F32, tag="xrc", bufs=3, name="xrc")
                    nc.sync.dma_start(out=xrc[:, :], in_=a2a_out[half][sm * P:(sm + 1) * P, :])
                    for k in range(4):
                        pt = ps.tile([P, P], F32, tag="pt", bufs=2, name="ptx")
                        nc.tensor.transpose(pt[:, :], xrc[:, k * P:(k + 1) * P], ident[:, :])
                        scopy(xsT[half * 4 + k][:, sm * P:(sm + 1) * P], pt[:, :])

            # expert FFN (transposed): gT/uT [FF, SR] tiles
            midT = [p_moe.tile([P, SR], F32R, name=f"midT{f}") for f in range(FT)]
            for fg in range(8):
                pg = [ps.tile([P, SR], F32, tag="big", bufs=4, name="pg") for _ in range(2)]
                pu = [ps.tile([P, SR], F32, tag="big", bufs=4, name="pu") for _ in range(2)]
                for k in range(DT_):
                    wgt = p_moe.tile([P, 256], F32R, tag="wgu", bufs=6, name="wgt")
                    nc.gpsimd.dma_start(out=wgt[:, :], in_=wg_p[k * P:(k + 1) * P, fg * 256:(fg + 1) * 256])
                    wut = p_moe.tile([P, 256], F32R, tag="wgu", bufs=6, name="wut")
                    nc.gpsimd.dma_start(out=wut[:, :], in_=wu_p[k * P:(k + 1) * P, fg * 256:(fg + 1) * 256])
                    for fm in range(2):
                        nc.tensor.matmul(pg[fm][:, :], wgt[:, fm * P:(fm + 1) * P],
                                         xsT[k][:, :],
                                         start=(k == 0), stop=(k == DT_ - 1))
                        nc.tensor.matmul(pu[fm][:, :], wut[:, fm * P:(fm + 1) * P],
                                         xsT[k][:, :],
                                         start=(k == 0), stop=(k == DT_ - 1))
                for fm in range(2):
                    gs = p_moe.tile([P, SR], F32, tag="gs", bufs=2, name="gs")
                    nc.scalar.activation(gs[:, :], pg[fm][:, :], AF.Silu)
                    nc.vector.tensor_tensor(midT[fg * 2 + fm][:, :], gs[:, :], pu[fm][:, :], OP.mult)

            # down: y_sel [SR, D], returned via chunked AllToAll
            for nt in range(2):
                pd = [ps.tile([P, 512], F32, tag="big", bufs=4, name="pd") for _ in range(SRT)]
                for k in range(FT):
                    wdt = p_moe.tile([P, 512], F32R, tag="wd512", bufs=6, name="wdt")
                    nc.gpsimd.dma_start(out=wdt[:, :], in_=wd_p[k * P:(k + 1) * P, nt * 512:(nt + 1) * 512])
                    for sm in range(SRT):
                        nc.tensor.matmul(pd[sm][:, :], midT[k][:, sm * P:(sm + 1) * P],
                                         wdt[:, :],
                                         start=(k == 0), stop=(k == FT - 1))
                for sm in range(SRT):
                    ys = p_moe.tile([P, 512], F32, tag="ysend", bufs=3, name="ys")
                    scopy(ys[:, :], pd[sm][:, :])
                    nc.sync.dma_start(out=bk_in[nt][sm * P:(sm + 1) * P, :], in_=ys[:, :])
                nc.gpsimd.collective_compute(
                    "AllToAll", OP.bypass, replica_groups=[list(range(NC_))],
                    ins=[bk_in[nt][:, :].opt()], outs=[bk_out[nt][:, :].opt()],
                )

            # un-sort: y_rows = PT_send @ y_back; output hidden = h2 + y_rows
            ohs = [p_moe.tile([P, D], F32, tag=f"ohs{j}", bufs=1, name=f"ohs{j}") for j in range(RT)]
            for j in range(RT):
                for nt in range(2):
                    py = ps.tile([P, 512], F32, tag="big", bufs=4, name="py")
                    for sm in range(SRT):
                        yb = p_moe.tile([P, 512], F32R, tag="yback", bufs=4, name="yb")
                        nc.gpsimd.dma_start(out=yb[:, :], in_=bk_out[nt][sm * P:(sm + 1) * P, :])
                        nc.tensor.matmul(py[:, :], PTT[sm][:, j * P:(j + 1) * P],
                                         yb[:, :],
                                         start=(sm == 0), stop=(sm == SRT - 1))
                    nc.vector.tensor_tensor(ohs[j][:, nt * 512:(nt + 1) * 512], py[:, :],
                                            h2[j][:, nt * 512:(nt + 1) * 512], OP.add)
                nc.sync.dma_start(out=oh_p[j * P:(j + 1) * P, :], in_=ohs[j][:, :])

            cm_moe.__exit__(None, None, None)

    nc.finalize()
    return nc


def _get_nc():
    if "nc" not in _CACHE:
        _CACHE["nc"] = _build()
    return _CACHE["nc"]


def _prep_in_maps(inputs):
    f32 = lambda a: np.ascontiguousarray(np.asarray(a), dtype=np.float32)
    hidden = f32(inputs["hidden"]); mu_prev = f32(inputs["mu_prev"]); velocity = f32(inputs["velocity"])
    positions = np.asarray(inputs["positions"]).astype(np.float32)
    token_ids = np.asarray(inputs["token_ids"])
    inv_freq = THETA ** (-np.arange(0, DH, 2, dtype=np.float32) / DH)
    ang = positions[:, None] * inv_freq
    cs = np.concatenate([np.cos(ang), np.sin(ang)], axis=1).astype(np.float32)  # [N, 64]
    base_ids = (token_ids % E).astype(np.int64)
    boh = (np.eye(E, dtype=np.float32)[base_ids] * BASE_SCALE).astype(np.float32)
    cowx = np.concatenate([f32(inputs["ctrl_out_w"]), f32(inputs["ctrl_out_b"])[None, :]], axis=0)
    shared = dict(
        wq=f32(inputs["wq"]), wmq=f32(inputs["w_mu_q"]),
        wk=f32(inputs["wk"]), wmk=f32(inputs["w_mu_k"]),
        wv=f32(inputs["wv"]), wmv=f32(inputs["w_mu_v"]),
        wo=f32(inputs["wo"]), dynw=f32(inputs["dyn_mu_proj_w"]),
        ciw=f32(inputs["ctrl_in_w"]), cib=f32(inputs["ctrl_in_b"])[None, :],
        cowx=cowx, mrw=f32(inputs["mu_router_w"]),
        ln1=f32(inputs["ln1_w"])[None, :], ln2=f32(inputs["ln2_w"])[None, :],
        qnw=np.tile(f32(inputs["qnorm_w"]), H)[None, :],
        knw=np.tile(f32(inputs["knorm_w"]), KV)[None, :],
        dmu=f32(inputs["dyn_mu"])[None, :],
        trib=np.triu(np.ones((P, P), np.float32), 1),
        ident=np.eye(P, dtype=np.float32),
        iotac=np.arange(C2, dtype=np.float32)[None, :],
        iota8=np.arange(E, dtype=np.float32)[None, :],
    )
    wg = f32(inputs["w_gate"]); wu = f32(inputs["w_up"]); wd = f32(inputs["w_down"])
    in_maps = []
    for c in range(NC_):
        sl = slice(c * NT, (c + 1) * NT)
        m = dict(shared)
        m.update(
            hid=hidden[sl], mu=mu_prev[sl], vel=velocity[sl],
            cs=cs[sl], boh=boh[sl],
            wg=np.ascontiguousarray(wg[c]), wu=np.ascontiguousarray(wu[c]),
            wd=np.ascontiguousarray(wd[c]),
        )
        in_maps.append(m)
    return in_maps, base_ids


def kernel(**inputs):
    nc = _get_nc()
    in_maps, base_ids = _prep_in_maps(inputs)
    res = run_bass_kernel_spmd(nc, in_maps, core_ids=list(range(NC_)))
    hidden = np.concatenate([res.results[c]["oh"] for c in range(NC_)], axis=0)
    v_next = np.concatenate([res.results[c]["ov"] for c in range(NC_)], axis=0)
    mu_cur = np.concatenate([res.results[c]["om"] for c in range(NC_)], axis=0)
    # dispatch-capacity sanity check (routing is dominated by the base one-hot:
    # margin ~10 vs mu logits ~0.05, and per-(src,dst) counts are Binom(256,1/8),
    # so C2=64 is a ~6-sigma bound)
    mrw = np.asarray(inputs["mu_router_w"], dtype=np.float32)
    logits = mu_cur @ mrw + np.eye(E, dtype=np.float32)[base_ids] * BASE_SCALE
    eids = logits.argmax(-1)
    for c in range(NC_):
        cnts = np.bincount(eids[c * NT:(c + 1) * NT], minlength=E)
        assert cnts.max() <= C2, f"dispatch capacity overflow on core {c}: {cnts}"
    return hidden, v_next, mu_cur


# revision 27
# speedup vs baseline: 13097.8968x; 12966.0722x over previous
"""Trainium2 Bass kernel for nn_ComplexityDecoderLayer (moe_routing).

Strategy (8 NeuronCores, SPMD):
  - Token-parallel attention + PID dynamics: each core owns 256 of 2048 tokens.
    K/V are computed per-shard (qk-norm + RoPE) and AllGathered in two
    head-pair chunks so attention on the first pair overlaps the second
    chunk's transfer.
  - Attention uses exp without max-subtraction (scores are O(1) after qk-norm)
    in transposed layout [keys, q]; softmax numerator and denominator come out
    of one PSUM accumulation via a ones-column appended to V.
  - Expert-parallel MoE with AllToAll token dispatch: each core sorts its own
    256 tokens by destination expert into an [8 x 64, 1024] send buffer using
    0/1 permutation matmuls built from a triangular-matmul prefix sum, then a
    2-chunk AllToAll delivers each expert its tokens; the expert FFN runs on
    the 512 received rows; results return via a second (chunked) AllToAll and
    are unsorted locally. No ReduceScatter needed.
All heavy compute is fp32 on device; the host only slices/concats inputs,
precomputes RoPE cos/sin tables from `positions` and the base-id one-hot from
`token_ids`, and reassembles the three outputs.
"""

import numpy as np

import concourse.mybir as mybir
import concourse.tile as tile
from concourse import bacc
from concourse.bass_utils import run_bass_kernel_spmd

F32 = mybir.dt.float32
F32R = mybir.dt.float32r
AF = mybir.ActivationFunctionType
OP = mybir.AluOpType
AX = mybir.AxisListType

P = 128
N, D, H, KV, DH, E, FF, CH = 2048, 1024, 16, 4, 64, 8, 2048, 64
NC_ = 8
NT = N // NC_          # 256 tokens per core
RT = NT // P           # 2 row tiles
DT_ = D // P           # 8
FT = FF // P           # 16
JT = N // P            # 16 global token tiles
C2 = 64                # per (src, dst) expert-dispatch capacity
SR = E * C2            # 512 rows through each expert
SRT = SR // P          # 4
EPS = 1e-6
THETA = 10000.0
DTC = 0.1
BASE_SCALE = 10.0

_CACHE = {}


def _build(r_attn=True):
    DT = F32R if r_attn else F32
    nc = bacc.Bacc(target_bir_lowering=False)

    def par(name, shp, dt=F32):
        return nc.declare_dram_parameter(name, list(shp), dt, isOutput=False)

    hid_p = par("hid", [NT, D])
    mu_p = par("mu", [NT, D])
    vel_p = par("vel", [NT, D])
    cs_p = par("cs", [NT, 2 * 32])          # [cos | sin]
    boh_p = par("boh", [NT, E])             # BASE_SCALE * one_hot(token_ids % E)
    wq_p = par("wq", [D, D], DT)
    wmq_p = par("wmq", [D, D], DT)
    wk_p = par("wk", [D, KV * DH], DT)
    wmk_p = par("wmk", [D, KV * DH], DT)
    wv_p = par("wv", [D, KV * DH], DT)
    wmv_p = par("wmv", [D, KV * DH], DT)
    wo_p = par("wo", [D, D], DT)
    dynw_p = par("dynw", [D, D], DT)
    ciw_p = par("ciw", [2 * D, CH], DT)
    cib_p = par("cib", [1, CH], DT)
    cowx_p = par("cowx", [CH + 1, 3 * D], DT)   # [ctrl_out_w ; ctrl_out_b]
    mrw_p = par("mrw", [D, E])
    wg_p = par("wg", [D, FF], F32R)               # this core's expert
    wu_p = par("wu", [D, FF], F32R)
    wd_p = par("wd", [FF, D], F32R)
    ln1_p = par("ln1", [1, D])
    ln2_p = par("ln2", [1, D])
    qnw_p = par("qnw", [1, D])              # qnorm_w tiled 16x
    knw_p = par("knw", [1, KV * DH])        # knorm_w tiled 4x
    dmu_p = par("dmu", [1, D], DT)
    trib_p = par("trib", [P, P])            # strict upper triangular ones
    ident_p = par("ident", [P, P])
    iotac_p = par("iotac", [1, C2])
    iota8_p = par("iota8", [1, E])
    ones_p = par("onesp", [1, NT], F32R)
    ones16_p = par("ones16", [1, JT], F32R)

    oh_p = nc.declare_dram_parameter("oh", [NT, D], F32, isOutput=True)
    ov_p = nc.declare_dram_parameter("ov", [NT, D], F32, isOutput=True)
    om_p = nc.declare_dram_parameter("om", [NT, D], F32, isOutput=True)

    with tile.TileContext(nc) as tc:
        from contextlib import ExitStack
        with ExitStack() as TOP:
            dram = TOP.enter_context(tc.tile_pool(name="dram", bufs=1, space="DRAM"))
            const = TOP.enter_context(tc.tile_pool(name="const", bufs=1))
            ps = TOP.enter_context(tc.tile_pool(name="ps", bufs=1, space="PSUM"))
            ws = TOP.enter_context(tc.tile_pool(name="wstream", bufs=1))
            work = TOP.enter_context(tc.tile_pool(name="work", bufs=1))
            top = TOP.enter_context(tc.tile_pool(name="top", bufs=1))

            # phase-scoped pools (manually closed LIFO per side to free SBUF)
            cm_dyn = tc.tile_pool(name="p_dyn", bufs=1); p_dyn = cm_dyn.__enter__()
            cm_att = tc.tile_pool(name="p_att", bufs=1); p_att = cm_att.__enter__()
            cm_hm = tc.tile_pool(name="p_hm", bufs=1); p_hm = cm_hm.__enter__()

            # ---------------- constants ----------------
            ident = const.tile([P, P], F32, name="identc")
            nc.sync.dma_start(out=ident[:, :], in_=ident_p[:, :])
            ident_r = const.tile([P, P], F32R, name="identr")
            nc.gpsimd.dma_start(out=ident_r[:, :], in_=ident_p[:, :])
            trib = const.tile([P, P], F32, name="tribc")
            nc.sync.dma_start(out=trib[:, :], in_=trib_p[:, :])
            iota64b = const.tile([P, C2], F32, name="iota64b")
            nc.sync.dma_start(out=iota64b[:, :], in_=iotac_p[:, :].to_broadcast((P, C2)))
            iota8b = const.tile([P, E], F32, name="iota8b")
            nc.sync.dma_start(out=iota8b[:, :], in_=iota8_p[:, :].to_broadcast((P, E)))
            epsb = const.tile([P, 1], F32, name="epsb")
            nc.vector.memset(epsb[:, :], EPS)
            ones_r = const.tile([1, NT], DT, name="onesr")   # lhsT row for bias matmuls
            nc.sync.dma_start(out=ones_r[:, :], in_=ones_p[:, :]) if DT == F32R else nc.vector.memset(ones_r[:, :], 1.0)
            ones_c = const.tile([P, 1], F32, name="onesc")    # rhs col for colsum
            nc.vector.memset(ones_c[:, :], 1.0)
            dmu_sb = const.tile([1, D], DT, name="dmusb")
            nc.sync.dma_start(out=dmu_sb[:, :], in_=dmu_p[:, :])
            cib_sb = const.tile([1, CH], DT, name="cibsb")
            nc.sync.dma_start(out=cib_sb[:, :], in_=cib_p[:, :])

            ln1b = p_dyn.tile([P, D], F32, name="ln1b")
            nc.sync.dma_start(out=ln1b[:, :], in_=ln1_p[:, :].to_broadcast((P, D)))
            ln2b = p_dyn.tile([P, D], F32, name="ln2b")
            nc.sync.dma_start(out=ln2b[:, :], in_=ln2_p[:, :].to_broadcast((P, D)))
            qnwb = p_dyn.tile([P, D], F32, name="qnwb")
            nc.sync.dma_start(out=qnwb[:, :], in_=qnw_p[:, :].to_broadcast((P, D)))
            knwb = p_dyn.tile([P, KV * DH], F32, name="knwb")
            nc.sync.dma_start(out=knwb[:, :], in_=knw_p[:, :].to_broadcast((P, KV * DH)))
            ciw_sb = p_dyn.tile([P, 16 * CH], DT, name="ciwsb")  # [2048,64] -> [128, 16*64]
            nc.sync.dma_start(
                out=ciw_sb[:, :].rearrange("p (j c) -> p j c", j=16),
                in_=ciw_p[:, :].rearrange("(j p) c -> p j c", p=P),
            )
            mrw_sb = p_dyn.tile([P, DT_ * E], F32, name="mrwsb")  # [1024,8] -> [128, 8*8]
            nc.sync.dma_start(
                out=mrw_sb[:, :].rearrange("p (j c) -> p j c", j=DT_),
                in_=mrw_p[:, :].rearrange("(j p) c -> p j c", p=P),
            )
            cos_sb = [p_dyn.tile([P, 32], F32, name=f"cos{rt}") for rt in range(RT)]
            sin_sb = [p_dyn.tile([P, 32], F32, name=f"sin{rt}") for rt in range(RT)]
            for rt in range(RT):
                nc.sync.dma_start(out=cos_sb[rt][:, :], in_=cs_p[rt * P:(rt + 1) * P, 0:32])
                nc.sync.dma_start(out=sin_sb[rt][:, :], in_=cs_p[rt * P:(rt + 1) * P, 32:64])

            # ---------------- DRAM internals ----------------
            # kv chunk i holds kv-head pair (2i, 2i+1): [k_pair | v_pair]
            kv_in = [dram.tile([NT, 2 * P], DT, name=f"kvin{i}") for i in range(2)]
            kv_full = [dram.tile([N, 2 * P], DT, name=f"kvfull{i}", addr_space="Shared") for i in range(2)]
            a2a_in = [dram.tile([SR, 512], F32R, name=f"a2ain{i}") for i in range(2)]
            a2a_out = [dram.tile([SR, 512], F32R, name=f"a2aout{i}") for i in range(2)]
            bk_in = [dram.tile([SR, 512], F32R, name=f"bkin{i}") for i in range(2)]
            bk_out = [dram.tile([SR, 512], F32R, name=f"bkout{i}") for i in range(2)]

            def peT(src_ap, dst_ap, engine, idt=None):
                """dst = src^T via PE transpose (src [p, f] -> dst [f, p])."""
                if idt is None:
                    idt = ident
                f = src_ap.shape[-1]
                p_ = src_ap.shape[0]
                pt = ps.tile([P, P], src_ap.dtype, tag="pt", bufs=2, name="pt")
                nc.tensor.transpose(pt[0:f, 0:p_], src_ap, idt[0:p_, 0:p_])
                engine(dst_ap, pt[0:f, 0:p_])

            vcopy = nc.vector.tensor_copy
            scopy = nc.scalar.copy

            def rmsnorm(dst, src, wb, ddim):
                t = work.tile([P, ddim], F32, tag="wk1024", bufs=3, name="rmst")
                sS = work.tile([P, 1], F32, tag="rms_s", bufs=4, name="rmss")
                nc.scalar.activation(t[:, 0:ddim], src, AF.Square, accum_out=sS[:, :])
                sq = work.tile([P, 1], F32, tag="rms_q", bufs=4, name="rmsq")
                nc.scalar.activation(sq[:, :], sS[:, :], AF.Sqrt, bias=epsb[:, :], scale=1.0 / ddim)
                rs_ = work.tile([P, 1], F32, tag="rms_r", bufs=4, name="rmsr")
                nc.vector.reciprocal(rs_[:, :], sq[:, :])
                nc.vector.tensor_scalar_mul(dst, src, rs_[:, :])
                nc.vector.tensor_tensor(dst, dst, wb, OP.mult)

            def headnorm(qr, nh, wb):
                for hh in range(nh):
                    sl = qr[:, hh * DH:(hh + 1) * DH]
                    t = work.tile([P, DH], F32, tag="hn_t", bufs=2, name="hnt")
                    sS = work.tile([P, 1], F32, tag="hn_s", bufs=4, name="hns")
                    nc.scalar.activation(t[:, :], sl, AF.Square, accum_out=sS[:, :])
                    sq = work.tile([P, 1], F32, tag="hn_q", bufs=4, name="hnq")
                    nc.scalar.activation(sq[:, :], sS[:, :], AF.Sqrt, bias=epsb[:, :], scale=1.0 / DH)
                    rs_ = work.tile([P, 1], F32, tag="hn_r", bufs=4, name="hnr")
                    nc.vector.reciprocal(rs_[:, :], sq[:, :])
                    nc.vector.tensor_scalar_mul(sl, sl, rs_[:, :])
                nc.vector.tensor_tensor(qr, qr, wb[:, 0:qr.shape[-1]], OP.mult)

            def rope(dst, src, rt, nh):
                s3 = src.rearrange("p (h d) -> p h d", h=nh)
                d3 = dst.rearrange("p (h d) -> p h d", h=nh)
                c3 = cos_sb[rt][:, :].rearrange("p (o d) -> p o d", o=1).to_broadcast((P, nh, 32))
                n3 = sin_sb[rt][:, :].rearrange("p (o d) -> p o d", o=1).to_broadcast((P, nh, 32))
                tmp = work.tile([P, H * 32], F32, tag="rope_t", bufs=1, name="ropet")
                t3 = tmp[:, 0:nh * 32].rearrange("p (h d) -> p h d", h=nh)
                x1 = s3[:, :, 0:32]
                x2 = s3[:, :, 32:64]
                nc.vector.tensor_tensor(d3[:, :, 0:32], x1, c3, OP.mult)
                nc.vector.tensor_tensor(t3, x2, n3, OP.mult)
                nc.vector.tensor_tensor(d3[:, :, 0:32], d3[:, :, 0:32], t3, OP.subtract)
                nc.vector.tensor_tensor(d3[:, :, 32:64], x2, c3, OP.mult)
                nc.vector.tensor_tensor(t3, x1, n3, OP.mult)
                nc.vector.tensor_tensor(d3[:, :, 32:64], d3[:, :, 32:64], t3, OP.add)

            # ================= Phase 1: h/mu transposes, k/v first =================
            hid = [p_dyn.tile([P, D], F32, name=f"hid{rt}") for rt in range(RT)]
            vel = [p_dyn.tile([P, D], F32, name=f"vel{rt}") for rt in range(RT)]
            velT = [p_dyn.tile([P, NT], DT, name=f"velT{k}") for k in range(DT_)]
            hT = [p_hm.tile([P, NT], DT, name=f"hT{k}") for k in range(DT_)]
            muT = [p_hm.tile([P, NT], DT, name=f"muT{k}") for k in range(DT_)]
            qrows = [p_hm.tile([P, D], F32, name=f"qrows{rt}") for rt in range(RT)]
            h2 = [top.tile([P, D], F32, name=f"h2{rt}") for rt in range(RT)]
            xr = [top.tile([P, D], F32R, name=f"xr{rt}") for rt in range(RT)]
            eid_loc = top.tile([P, RT], F32, name="eidloc")

            for rt in range(RT):
                nc.sync.dma_start(out=hid[rt][:, :], in_=hid_p[rt * P:(rt + 1) * P, :])
                h = work.tile([P, D], F32, tag="wk1024", bufs=3, name="hrows")
                rmsnorm(h[:, :], hid[rt][:, :], ln1b[:, :], D)
                mrow = work.tile([P, D], F32, tag="wk1024", bufs=3, name="murows")
                nc.sync.dma_start(out=mrow[:, :], in_=mu_p[rt * P:(rt + 1) * P, :])
                for k in range(DT_):
                    peT(h[:, k * P:(k + 1) * P], hT[k][:, rt * P:(rt + 1) * P], vcopy)
                    peT(mrow[:, k * P:(k + 1) * P], muT[k][:, rt * P:(rt + 1) * P], vcopy)

            # k/v rows first so the kv AllGathers overlap the q-side work
            for rt in range(RT):
                pk = ps.tile([P, KV * DH], F32, tag="big", bufs=4, name="pk")
                pv = ps.tile([P, KV * DH], F32, tag="big", bufs=4, name="pv")
                i = 0
                for lhsT, wp1, wp2 in ((hT, wk_p, wv_p), (muT, wmk_p, wmv_p)):
                    for k in range(DT_):
                        wt1 = ws.tile([P, KV * DH], DT, tag="w256", bufs=4, name="wt1")
                        nc.sync.dma_start(out=wt1[:, :], in_=wp1[k * P:(k + 1) * P, :])
                        wt2 = ws.tile([P, KV * DH], DT, tag="w256", bufs=4, name="wt2")
                        nc.sync.dma_start(out=wt2[:, :], in_=wp2[k * P:(k + 1) * P, :])
                        nc.tensor.matmul(pk[:, :], lhsT[k][:, rt * P:(rt + 1) * P],
                                         wt1[:, :], start=(i == 0), stop=(i == 2 * DT_ - 1))
                        nc.tensor.matmul(pv[:, :], lhsT[k][:, rt * P:(rt + 1) * P],
                                         wt2[:, :], start=(i == 0), stop=(i == 2 * DT_ - 1))
                        i += 1
                krow = p_hm.tile([P, KV * DH], F32, tag="kv256", bufs=2, name="krow")
                vrow = p_hm.tile([P, KV * DH], DT, tag="kv256b", bufs=2, name="vrow")
                vcopy(krow[:, :], pk[:, :])
                vcopy(vrow[:, :], pv[:, :])
                headnorm(krow[:, :], KV, knwb)
                rk = p_hm.tile([P, KV * DH], DT, tag="kv256c", bufs=2, name="rk")
                rope(rk[:, :], krow[:, :], rt, KV)
                for i in range(2):
                    nc.sync.dma_start(out=kv_in[i][rt * P:(rt + 1) * P, 0:P], in_=rk[:, i * P:(i + 1) * P])
                    nc.sync.dma_start(out=kv_in[i][rt * P:(rt + 1) * P, P:2 * P], in_=vrow[:, i * P:(i + 1) * P])

            for i in range(2):
                nc.gpsimd.collective_compute(
                    "AllGather", OP.bypass, replica_groups=[list(range(NC_))],
                    ins=[kv_in[i][:, :].opt()], outs=[kv_full[i][:, :].opt()],
                )

            # q rows = h @ wq + mu @ wmq (overlaps kv AllGathers)
            for nt in range(2):
                pq = [ps.tile([P, 512], F32, tag="big", bufs=4, name="pq") for _ in range(RT)]
                i = 0
                for lhsT, w_p in ((hT, wq_p), (muT, wmq_p)):
                    for k in range(DT_):
                        wt = ws.tile([P, 512], DT, tag="w512", bufs=3, name="wt")
                        nc.sync.dma_start(out=wt[:, :], in_=w_p[k * P:(k + 1) * P, nt * 512:(nt + 1) * 512])
                        for rt in range(RT):
                            nc.tensor.matmul(pq[rt][:, :], lhsT[k][:, rt * P:(rt + 1) * P], wt[:, :],
                                             start=(i == 0), stop=(i == 2 * DT_ - 1))
                        i += 1
                for rt in range(RT):
                    vcopy(qrows[rt][:, nt * 512:(nt + 1) * 512], pq[rt][:, :])

            qT = [p_att.tile([DH, NT], DT, name=f"qT{hh}") for hh in range(H)]
            for rt in range(RT):
                headnorm(qrows[rt][:, :], H, qnwb)
                rq = work.tile([P, D], F32, tag="wk1024", bufs=3, name="rq")
                rope(rq[:, :], qrows[rt][:, :], rt, H)
                for k in range(DT_):
                    pt = ps.tile([P, P], F32, tag="pt", bufs=2, name="ptq")
                    nc.tensor.transpose(pt[:, :], rq[:, k * P:(k + 1) * P], ident[:, :])
                    vcopy(qT[2 * k][:, rt * P:(rt + 1) * P], pt[0:DH, :])
                    vcopy(qT[2 * k + 1][:, rt * P:(rt + 1) * P], pt[DH:P, :])

            cm_hm.__exit__(None, None, None)  # free hT/muT/qrows

            # ================= Phase 2+3: unpack k/v per chunk; attention =================
            cm_o = tc.tile_pool(name="p_o", bufs=1, side="right"); p_o = cm_o.__enter__()
            oT = [p_o.tile([P, NT], DT, name=f"oT{k}") for k in range(DT_)]
            kT = [p_att.tile([DH, N], DT, name=f"kT{g}") for g in range(KV)]
            vext4 = [p_att.tile([P, JT * 65], DT, name=f"vext{g}") for g in range(KV)]
            for chunk in range(2):
                for g in (2 * chunk, 2 * chunk + 1):
                    if DT == F32R:
                        nc.sync.dma_start(
                            out=vext4[g][:, :].rearrange("p (t c) -> p t c", c=65)[:, :, 64:65],
                            in_=ones16_p[:, :].rearrange("o (t c) -> o t c", c=1).to_broadcast((P, JT, 1)),
                        )
                    else:
                        nc.vector.memset(vext4[g][:, :], 1.0)
                for hf in range(2):
                    klb = p_att.tile([P, N // 2], DT, tag="klb", bufs=1, name="klb")
                    nc.sync.dma_start(
                        out=klb[:, :].rearrange("p (t c) -> p t c", c=P),
                        in_=kv_full[chunk][hf * 1024:(hf + 1) * 1024, 0:P].rearrange("(t p) c -> p t c", p=P),
                    )
                    for t2 in range(JT // 2):
                        tt = hf * 8 + t2
                        pt = ps.tile([P, P], DT, tag="pt", bufs=2, name="ptk")
                        nc.tensor.transpose(pt[:, :], klb[:, t2 * P:(t2 + 1) * P], ident_r[:, :] if r_attn else ident[:, :])
                        vcopy(kT[2 * chunk][:, tt * P:(tt + 1) * P], pt[0:DH, :])
                        vcopy(kT[2 * chunk + 1][:, tt * P:(tt + 1) * P], pt[DH:P, :])
                for hf in range(2):
                    for gl in range(2):
                        g = 2 * chunk + gl
                        nc.sync.dma_start(
                            out=vext4[g][:, hf * 8 * 65:(hf + 1) * 8 * 65].rearrange("p (t c) -> p t c", c=65)[:, :, 0:64],
                            in_=kv_full[chunk][hf * 1024:(hf + 1) * 1024, P + gl * DH:P + (gl + 1) * DH].rearrange("(t p) c -> p t c", p=P),
                        )
                for g in (2 * chunk, 2 * chunk + 1):
                    vext = vext4[g]
                    for hq in range(H // KV):
                        hh = g * (H // KV) + hq
                        qTh = qT[hh][:, :]
                        pO = ps.tile([65, NT], F32, tag="oext", bufs=2, name="pO")
                        for tt in range(JT):
                            pS = ps.tile([P, NT], F32, tag="big", bufs=4, name="pS")
                            nc.tensor.matmul(pS[:, :], kT[g][:, tt * P:(tt + 1) * P],
                                             qTh, start=True, stop=True)
                            ex = p_att.tile([P, NT], DT, tag="ex", bufs=2, name="ex")
                            nc.scalar.activation(ex[:, :], pS[:, :], AF.Exp, scale=0.125)
                            nc.tensor.matmul(pO[:, :], vext[:, tt * 65:(tt + 1) * 65], ex[:, :],
                                             start=(tt == 0), stop=(tt == JT - 1))
                        rd = p_att.tile([1, NT], F32, tag="rd", bufs=2, name="rd")
                        nc.vector.reciprocal(rd[:, :], pO[64:65, :])
                        rdb = p_att.tile([DH, NT], F32, tag="rdb", bufs=2, name="rdb")
                        nc.gpsimd.partition_broadcast(rdb[:, :], rd[:, :])
                        nc.vector.tensor_tensor(oT[hh // 2][(hh % 2) * DH:(hh % 2 + 1) * DH, :],
                                                pO[0:DH, :], rdb[:, :], OP.mult)

            cm_att.__exit__(None, None, None)  # free qT/kT/vext

            # ================= Phase 4: wo + dynamics + router =================
            cm_wo = tc.tile_pool(name="p_wo", bufs=1); p_wo = cm_wo.__enter__()
            orows = [p_wo.tile([P, D], F32, name=f"orows{rt}") for rt in range(RT)]
            for nt in range(2):
                po = [ps.tile([P, 512], F32, tag="big", bufs=4, name="po") for _ in range(RT)]
                for k in range(DT_):
                    wt = ws.tile([P, 512], DT, tag="w512", bufs=3, name="wot")
                    nc.sync.dma_start(out=wt[:, :], in_=wo_p[k * P:(k + 1) * P, nt * 512:(nt + 1) * 512])
                    for rt in range(RT):
                        nc.tensor.matmul(po[rt][:, :], oT[k][:, rt * P:(rt + 1) * P], wt[:, :],
                                         start=(k == 0), stop=(k == DT_ - 1))
                for rt in range(RT):
                    vcopy(orows[rt][:, nt * 512:(nt + 1) * 512], po[rt][:, :])

            oTw = [p_wo.tile([P, NT], DT, name=f"oTw{k}") for k in range(DT_)]
            for rt in range(RT):
                for k in range(DT_):
                    peT(orows[rt][:, k * P:(k + 1) * P], oTw[k][:, rt * P:(rt + 1) * P], vcopy)
            cm_o.__exit__(None, None, None)  # free oT

            # mu_cur = dyn_mu + o @ dynw
            mucur = [p_wo.tile([P, D], F32, name=f"mucur{rt}") for rt in range(RT)]
            for nt in range(2):
                pm = [ps.tile([P, 512], F32, tag="big", bufs=4, name="pm") for _ in range(RT)]
                for k in range(DT_):
                    wt = ws.tile([P, 512], DT, tag="w512", bufs=3, name="dynt")
                    nc.sync.dma_start(out=wt[:, :], in_=dynw_p[k * P:(k + 1) * P, nt * 512:(nt + 1) * 512])
                    for rt in range(RT):
                        nc.tensor.matmul(pm[rt][:, :], oTw[k][:, rt * P:(rt + 1) * P], wt[:, :],
                                         start=(k == 0), stop=False)
                for rt in range(RT):
                    nc.tensor.matmul(pm[rt][:, :], ones_r[0:1, rt * P:(rt + 1) * P],
                                     dmu_sb[0:1, nt * 512:(nt + 1) * 512], start=False, stop=True)
                    vcopy(mucur[rt][:, nt * 512:(nt + 1) * 512], pm[rt][:, :])
            for rt in range(RT):
                nc.sync.dma_start(out=om_p[rt * P:(rt + 1) * P, :], in_=mucur[rt][:, :])

            # router early: eid depends only on mu_cur
            mcT = [p_wo.tile([P, NT], F32, name=f"mcT{k}") for k in range(DT_)]
            for rt in range(RT):
                for k in range(DT_):
                    peT(mucur[rt][:, k * P:(k + 1) * P], mcT[k][:, rt * P:(rt + 1) * P], vcopy)
            for rt in range(RT):
                pr = ps.tile([P, E], F32, tag="big", bufs=4, name="pr")
                for k in range(DT_):
                    nc.tensor.matmul(pr[:, :], mcT[k][:, rt * P:(rt + 1) * P],
                                     mrw_sb[:, k * E:(k + 1) * E], start=(k == 0), stop=(k == DT_ - 1))
                cmb = work.tile([P, E], F32, tag="cmb", bufs=2, name="cmb")
                bohs = work.tile([P, E], F32, tag="bohs", bufs=2, name="bohs")
                nc.sync.dma_start(out=bohs[:, :], in_=boh_p[rt * P:(rt + 1) * P, :])
                nc.vector.tensor_tensor(cmb[:, :], pr[:, :], bohs[:, :], OP.add)
                mx = work.tile([P, 1], F32, tag="mx", bufs=2, name="mx")
                nc.vector.reduce_max(mx[:, :], cmb[:, :], axis=AX.X)
                nc.vector.tensor_scalar(cmb[:, :], cmb[:, :], mx[:, :], None, OP.is_equal)
                nc.vector.tensor_tensor(cmb[:, :], cmb[:, :], iota8b[:, :], OP.mult)
                nc.vector.reduce_sum(eid_loc[:, rt:rt + 1], cmb[:, :], axis=AX.X)

            # ctrl MLP (velocity loaded/transposed here, off the critical path)
            for rt in range(RT):
                nc.sync.dma_start(out=vel[rt][:, :], in_=vel_p[rt * P:(rt + 1) * P, :])
            for rt in range(RT):
                for k in range(DT_):
                    peT(vel[rt][:, k * P:(k + 1) * P], velT[k][:, rt * P:(rt + 1) * P], vcopy)
            ctT = p_wo.tile([CH + 1, NT], DT, name="ctT")
            if DT == F32R:
                nc.sync.dma_start(out=ctT[CH:CH + 1, :], in_=ones_p[:, :])
            else:
                nc.vector.memset(ctT[CH:CH + 1, :], 1.0)
            for rt in range(RT):
                pc = ps.tile([P, CH], F32, tag="big", bufs=4, name="pc")
                for k in range(DT_):
                    nc.tensor.matmul(pc[:, :], oTw[k][:, rt * P:(rt + 1) * P],
                                     ciw_sb[:, k * CH:(k + 1) * CH], start=(k == 0), stop=False)
                for k in range(DT_):
                    nc.tensor.matmul(pc[:, :], velT[k][:, rt * P:(rt + 1) * P],
                                     ciw_sb[:, (DT_ + k) * CH:(DT_ + k + 1) * CH], start=False, stop=False)
                nc.tensor.matmul(pc[:, :], ones_r[0:1, rt * P:(rt + 1) * P], cib_sb[0:1, :],
                                 start=False, stop=True)
                ct = work.tile([P, CH], F32, tag="ct", bufs=2, name="ct")
                nc.scalar.activation(ct[:, :], pc[:, :], AF.Silu)
                peT(ct[:, :], ctT[0:CH, rt * P:(rt + 1) * P], vcopy)

            abg = [[p_wo.tile([P, D], F32, name=f"abg{i}{rt}") for rt in range(RT)] for i in range(3)]
            for nt in (0, 1, 4, 5, 2, 3):
                cw = ws.tile([CH + 1, 512], DT, tag="cow", bufs=2, name="cw")
                nc.sync.dma_start(out=cw[:, :], in_=cowx_p[:, nt * 512:(nt + 1) * 512])
                for rt in range(RT):
                    pb = ps.tile([P, 512], F32, tag="big", bufs=4, name="pb")
                    nc.tensor.matmul(pb[:, :], ctT[:, rt * P:(rt + 1) * P], cw[:, :],
                                     start=True, stop=True)
                    dst = abg[nt // 2][rt][:, (nt % 2) * 512:(nt % 2 + 1) * 512]
                    if nt // 2 != 1:
                        nc.scalar.activation(dst, pb[:, :], AF.Sigmoid)
                    else:
                        # softplus = ln(1 + exp(x)); Exp/Ln share one ACT table.
                        # exp overflow -> inf -> ln -> inf -> min(.,2) still correct.
                        nc.scalar.activation(dst, pb[:, :], AF.Exp)
                        nc.vector.tensor_scalar_add(dst, dst, 1.0)
                        nc.scalar.activation(dst, dst, AF.Ln)
            for rt in range(RT):
                nc.vector.tensor_scalar_min(abg[1][rt][:, :], abg[1][rt][:, :], 2.0)

            # dynamics elementwise + x
            for rt in range(RT):
                err = work.tile([P, D], F32, tag="wk1024", bufs=3, name="err")
                nc.vector.tensor_tensor(err[:, :], orows[rt][:, :], mucur[rt][:, :], OP.subtract)
                av = work.tile([P, D], F32, tag="wk1024", bufs=3, name="av")
                nc.vector.tensor_tensor(av[:, :], abg[0][rt][:, :], vel[rt][:, :], OP.mult)
                nc.vector.tensor_tensor(err[:, :], abg[1][rt][:, :], err[:, :], OP.mult)
                nc.vector.tensor_tensor(av[:, :], av[:, :], err[:, :], OP.subtract)
                nc.vector.tensor_scalar_min(av[:, :], av[:, :], 10.0)
                nc.vector.tensor_scalar_max(av[:, :], av[:, :], -10.0)
                nc.sync.dma_start(out=ov_p[rt * P:(rt + 1) * P, :], in_=av[:, :])
                gv = work.tile([P, D], F32, tag="wk1024", bufs=3, name="gv")
                nc.vector.tensor_tensor(gv[:, :], abg[2][rt][:, :], av[:, :], OP.mult)
                nc.vector.tensor_scalar_mul(gv[:, :], gv[:, :], DTC)
                nc.vector.tensor_tensor(gv[:, :], gv[:, :], orows[rt][:, :], OP.add)
                nc.vector.tensor_tensor(h2[rt][:, :], gv[:, :], hid[rt][:, :], OP.add)
                rmsnorm(xr[rt][:, :], h2[rt][:, :], ln2b[:, :], D)

            cm_wo.__exit__(None, None, None)   # free orows/oTw/mucur/ctT/abg/mcT
            cm_dyn.__exit__(None, None, None)  # free hid/vel/velT/phase consts

            # ================= Phase 5: local sort + AllToAll MoE =================
            cm_moe = tc.tile_pool(name="p_moe", bufs=1); p_moe = cm_moe.__enter__()
            # PT_send [256 tok, 8*C2]: PT[t, d*C2+s] = 1 iff token t is the s-th
            # token (in order) among this core's tokens routed to expert d
            PTs = [p_moe.tile([P, SR], F32R, name=f"PTs{j}") for j in range(RT)]
            for d in range(E):
                maskd = p_moe.tile([P, RT], F32, tag="maskd", bufs=2, name="maskd")
                nc.vector.tensor_scalar(maskd[:, :], eid_loc[:, :], float(d), None, OP.is_equal)
                pexl = ps.tile([P, RT], F32, tag="pt", bufs=2, name="pexl")
                nc.tensor.matmul(pexl[:, :], trib[:, :], maskd[:, :], start=True, stop=True)
                pcs2 = ps.tile([RT, 1], F32, tag="pt", bufs=2, name="pcs2")
                nc.tensor.matmul(pcs2[:, :], maskd[:, :], ones_c[:, :], start=True, stop=True)
                cs2 = p_moe.tile([RT, 1], F32, tag="cs2", bufs=2, name="cs2")
                vcopy(cs2[:, :], pcs2[:, :])
                csb0 = p_moe.tile([P, 1], F32, tag="csb0", bufs=2, name="csb0")
                nc.gpsimd.partition_broadcast(csb0[:, :], cs2[0:1, 0:1])
                rankd = p_moe.tile([P, RT], F32, tag="rankd", bufs=2, name="rankd")
                vcopy(rankd[:, 0:1], pexl[:, 0:1])
                nc.vector.tensor_tensor(rankd[:, 1:2], pexl[:, 1:2], csb0[:, :], OP.add)
                for j in range(RT):
                    nc.vector.tensor_scalar(PTs[j][:, d * C2:(d + 1) * C2], iota64b[:, :],
                                            rankd[:, j:j + 1], maskd[:, j:j + 1],
                                            OP.is_equal, OP.mult)
            # x_send = PT_send^T @ x_rows -> [SR, 1024], sent as 2 column chunks
            for half in range(2):
                for sm in range(SRT):
                    pxs = ps.tile([P, 512], F32, tag="big", bufs=4, name="pxs")
                    for j in range(RT):
                        nc.tensor.matmul(pxs[:, :], PTs[j][:, sm * P:(sm + 1) * P],
                                         xr[j][:, half * 512:(half + 1) * 512],
                                         start=(j == 0), stop=(j == RT - 1))
                    xs = p_moe.tile([P, 512], F32R, tag="xsend", bufs=3, name="xs")
                    scopy(xs[:, :], pxs[:, :])
                    nc.sync.dma_start(out=a2a_in[half][sm * P:(sm + 1) * P, :], in_=xs[:, :])
                nc.gpsimd.collective_compute(
                    "AllToAll", OP.bypass, replica_groups=[list(range(NC_))],
                    ins=[a2a_in[half][:, :].opt()], outs=[a2a_out[half][:, :].opt()],
                )
            # PT_send^T for the un-sort at the end
            PTT = [p_moe.tile([P, NT], F32R, name=f"PTT{sm}") for sm in range(SRT)]
            for j in range(RT):
                for sm in range(SRT):
                    peT(PTs[j][:, sm * P:(sm + 1) * P], PTT[sm][:, j * P:(j + 1) * P], scopy, idt=ident_r)

            # received tokens -> transposed activations xsT [1024, SR]
            xsT = [p_moe.tile([P, SR], F32R, name=f"xsT{k}") for k in range(DT_)]
            for half in range(2):
                for sm in range(SRT):
                    xrc = p_moe.tile([P, 512], F32R, tag="xrc", bufs=3, name="xrc")
                    nc.sync.dma_start(out=xrc[:, :], in_=a2a_out[half][sm * P:(sm + 1) * P, :])
                    for k in range(4):
                        pt = ps.tile([P, P], F32R, tag="pt", bufs=2, name="ptx")
                        nc.tensor.transpose(pt[:, :], xrc[:, k * P:(k + 1) * P], ident_r[:, :])
                        scopy(xsT[half * 4 + k][:, sm * P:(sm + 1) * P], pt[:, :])

            # expert FFN (transposed): gT/uT [FF, SR] tiles
            midT = [p_moe.tile([P, SR], F32R, name=f"midT{f}") for f in range(FT)]
            for fg in range(4):
                pg = [ps.tile([P, SR], F32, tag=t, bufs=b, name="pg")
                      for t, b in (("big", 4), ("big", 4), ("oext", 2), ("oext", 2))]
                pu = [ps.tile([P, SR], F32, tag=t, bufs=b, name="pu")
                      for t, b in (("big", 4), ("big", 4), ("pt", 2), ("pt", 2))]
                for k in range(DT_):
                    wgt = p_moe.tile([P, 512], F32R, tag="wgu", bufs=3, name="wgt")
                    nc.sync.dma_start(out=wgt[:, :], in_=wg_p[k * P:(k + 1) * P, fg * 512:(fg + 1) * 512])
                    wut = p_moe.tile([P, 512], F32R, tag="wgu2", bufs=3, name="wut")
                    nc.sync.dma_start(out=wut[:, :], in_=wu_p[k * P:(k + 1) * P, fg * 512:(fg + 1) * 512])
                    for fm in range(4):
                        nc.tensor.matmul(pg[fm][:, :], wgt[:, fm * P:(fm + 1) * P],
                                         xsT[k][:, :],
                                         start=(k == 0), stop=(k == DT_ - 1))
                        nc.tensor.matmul(pu[fm][:, :], wut[:, fm * P:(fm + 1) * P],
                                         xsT[k][:, :],
                                         start=(k == 0), stop=(k == DT_ - 1))
                for fm in range(4):
                    gs = p_moe.tile([P, SR], F32, tag="gs", bufs=2, name="gs")
                    nc.scalar.activation(gs[:, :], pg[fm][:, :], AF.Silu)
                    nc.vector.tensor_tensor(midT[fg * 4 + fm][:, :], gs[:, :], pu[fm][:, :], OP.mult)

            # down: y_sel [SR, D], returned via chunked AllToAll
            for nt in range(2):
                pd = [ps.tile([P, 512], F32, tag="big", bufs=4, name="pd") for _ in range(SRT)]
                for k in range(FT):
                    wdt = p_moe.tile([P, 512], F32R, tag="wd512", bufs=6, name="wdt")
                    nc.sync.dma_start(out=wdt[:, :], in_=wd_p[k * P:(k + 1) * P, nt * 512:(nt + 1) * 512])
                    for sm in range(SRT):
                        nc.tensor.matmul(pd[sm][:, :], midT[k][:, sm * P:(sm + 1) * P],
                                         wdt[:, :],
                                         start=(k == 0), stop=(k == FT - 1))
                for sm in range(SRT):
                    ys = p_moe.tile([P, 512], F32R, tag="ysend", bufs=3, name="ys")
                    scopy(ys[:, :], pd[sm][:, :])
                    nc.sync.dma_start(out=bk_in[nt][sm * P:(sm + 1) * P, :], in_=ys[:, :])
                nc.gpsimd.collective_compute(
                    "AllToAll", OP.bypass, replica_groups=[list(range(NC_))],
                    ins=[bk_in[nt][:, :].opt()], outs=[bk_out[nt][:, :].opt()],
                )

            # un-sort: y_rows = PT_send @ y_back; output hidden = h2 + y_rows
            ohs = [p_moe.tile([P, D], F32, tag=f"ohs{j}", bufs=1, name=f"ohs{j}") for j in range(RT)]
            for j in range(RT):
                for nt in range(2):
                    py = ps.tile([P, 512], F32, tag="big", bufs=4, name="py")
                    for sm in range(SRT):
                        yb = p_moe.tile([P, 512], F32R, tag="yback", bufs=4, name="yb")
                        nc.sync.dma_start(out=yb[:, :], in_=bk_out[nt][sm * P:(sm + 1) * P, :])
                        nc.tensor.matmul(py[:, :], PTT[sm][:, j * P:(j + 1) * P],
                                         yb[:, :],
                                         start=(sm == 0), stop=(sm == SRT - 1))
                    nc.vector.tensor_tensor(ohs[j][:, nt * 512:(nt + 1) * 512], py[:, :],
                                            h2[j][:, nt * 512:(nt + 1) * 512], OP.add)
                nc.sync.dma_start(out=oh_p[j * P:(j + 1) * P, :], in_=ohs[j][:, :])

            cm_moe.__exit__(None, None, None)

    nc.finalize()
    return nc


import os
R_ATTN = os.environ.get("KERNEL_F32R_ATTN", "1") == "1"


def _get_nc():
    key = ("nc", R_ATTN)
    if key not in _CACHE:
        _CACHE[key] = _build(R_ATTN)
    return _CACHE[key]


def _prep_in_maps(inputs):
    f32 = lambda a: np.ascontiguousarray(np.asarray(a), dtype=np.float32)
    hidden = f32(inputs["hidden"]); mu_prev = f32(inputs["mu_prev"]); velocity = f32(inputs["velocity"])
    positions = np.asarray(inputs["positions"]).astype(np.float32)
    token_ids = np.asarray(inputs["token_ids"])
    inv_freq = THETA ** (-np.arange(0, DH, 2, dtype=np.float32) / DH)
    ang = positions[:, None] * inv_freq
    cs = np.concatenate([np.cos(ang), np.sin(ang)], axis=1).astype(np.float32)  # [N, 64]
    base_ids = (token_ids % E).astype(np.int64)
    boh = (np.eye(E, dtype=np.float32)[base_ids] * BASE_SCALE).astype(np.float32)
    cowx = np.concatenate([f32(inputs["ctrl_out_w"]), f32(inputs["ctrl_out_b"])[None, :]], axis=0)
    shared = dict(
        wq=f32(inputs["wq"]), wmq=f32(inputs["w_mu_q"]),
        wk=f32(inputs["wk"]), wmk=f32(inputs["w_mu_k"]),
        wv=f32(inputs["wv"]), wmv=f32(inputs["w_mu_v"]),
        wo=f32(inputs["wo"]), dynw=f32(inputs["dyn_mu_proj_w"]),
        ciw=f32(inputs["ctrl_in_w"]), cib=f32(inputs["ctrl_in_b"])[None, :],
        cowx=cowx, mrw=f32(inputs["mu_router_w"]),
        ln1=f32(inputs["ln1_w"])[None, :], ln2=f32(inputs["ln2_w"])[None, :],
        qnw=np.tile(f32(inputs["qnorm_w"]), H)[None, :],
        knw=np.tile(f32(inputs["knorm_w"]), KV)[None, :],
        dmu=f32(inputs["dyn_mu"])[None, :],
        trib=np.triu(np.ones((P, P), np.float32), 1),
        onesp=np.ones((1, NT), np.float32),
        ones16=np.ones((1, JT), np.float32),
        ident=np.eye(P, dtype=np.float32),
        iotac=np.arange(C2, dtype=np.float32)[None, :],
        iota8=np.arange(E, dtype=np.float32)[None, :],
    )
    wg = f32(inputs["w_gate"]); wu = f32(inputs["w_up"]); wd = f32(inputs["w_down"])
    in_maps = []
    for c in range(NC_):
        sl = slice(c * NT, (c + 1) * NT)
        m = dict(shared)
        m.update(
            hid=hidden[sl], mu=mu_prev[sl], vel=velocity[sl],
            cs=cs[sl], boh=boh[sl],
            wg=np.ascontiguousarray(wg[c]), wu=np.ascontiguousarray(wu[c]),
            wd=np.ascontiguousarray(wd[c]),
        )
        in_maps.append(m)
    return in_maps, base_ids


def kernel(**inputs):
    nc = _get_nc()
    in_maps, base_ids = _prep_in_maps(inputs)
    res = run_bass_kernel_spmd(nc, in_maps, core_ids=list(range(NC_)))
    hidden = np.concatenate([res.results[c]["oh"] for c in range(NC_)], axis=0)
    v_next = np.concatenate([res.results[c]["ov"] for c in range(NC_)], axis=0)
    mu_cur = np.concatenate([res.results[c]["om"] for c in range(NC_)], axis=0)
    # dispatch-capacity sanity check (routing is dominated by the base one-hot:
    # margin ~10 vs mu logits ~0.05, and per-(src,dst) counts are Binom(256,1/8),
    # so C2=64 is a ~6-sigma bound)
    mrw = np.asarray(inputs["mu_router_w"], dtype=np.float32)
    logits = mu_cur @ mrw + np.eye(E, dtype=np.float32)[base_ids] * BASE_SCALE
    eids = logits.argmax(-1)
    for c in range(NC_):
        cnts = np.bincount(eids[c * NT:(c + 1) * NT], minlength=E)
        assert cnts.max() <= C2, f"dispatch capacity overflow on core {c}: {cnts}"
    return hidden, v_next, mu_cur
